# revision 11
# baseline (speedup 1.0000x reference)
"""Trainium2 Bass kernel for nn_AffectChannel (compress + GELU + 16-dim GRU scan).

Strategy (8 NeuronCores, data-parallel over batch, one batch element per core):
  Phase 1 (memory-bound): compressed = gelu(residual @ compress_w.T + b)
    - residual shard is pre-transposed on host -> fully coalesced DMA, fp32
      matmuls contract d on partitions, accumulate in PSUM.
  Phase 2: x_gates = compressed @ w_ih.T + biases, written in a "blocked"
    layout: partitions = 16 hidden lanes x 8 time-blocks (512 steps each).
  Phase 3: the sequential GRU scan is computed by Picard iteration: gates are
    evaluated from the previous trajectory estimate (fully parallel, 128-lane
    ops), then the diagonal blend recurrence h_t = z_t h_{t-1} + (1-z_t) n_t
    is solved EXACTLY with the DVE tensor_tensor_scan instruction (per-block
    prefix scans + an 8-block boundary chain via tiny PE gather/scatter
    matmuls).  ~24 sweeps converge to fp32 accuracy (contraction ~0.5/sweep).
"""
import json
import os

import numpy as np

B, S, D, C, H = 8, 4096, 2048, 64, 16
NB = 8           # time blocks
L = S // NB      # block length = 512
NCORES = 8
N_SWEEP = int(os.environ.get("AFFECT_N_SWEEP", "24"))


# --- walrus workaround: split multi-wait instructions ----------------------
def _split_multiwaits(d):
    n = 0
    uid = [0]
    for f in d.get("functions", []):
        for blk in f.get("blocks", []):
            out = []
            for ins in blk.get("instructions", []):
                si = ins.get("sync_info")
                waits = (si or {}).get("on_wait") or []
                if len(waits) > 1:
                    n += 1
                    for w in waits[:-1]:
                        uid[0] += 1
                        out.append({
                            "opcode": "EventSemaphore",
                            "name": f"{ins['name']}_wsplit{uid[0]}",
                            "engine": ins["engine"],
                            "ins": [], "outs": [],
                            "debug": ins.get("debug"),
                            "sync_info": {"on_wait": [w], "on_update": []},
                        })
                    si["on_wait"] = [waits[-1]]
                out.append(ins)
            blk["instructions"] = out
    return n


def _fix_bir_json(bir_json):
    if isinstance(bir_json, str):
        bir_json = bir_json.encode()
    d = json.loads(bir_json)
    if _split_multiwaits(d) == 0:
        return bir_json
    return json.dumps(d).encode()


_PATCHED = False


def _install_bir_fix():
    global _PATCHED
    if _PATCHED:
        return
    _PATCHED = True
    import concourse.bass_utils as bu
    import concourse.bass2jax as b2j

    orig = bu.compile_bir_kernel

    def patched(bir_json, tmpdir, neff_name="file.neff"):
        return orig(_fix_bir_json(bir_json), tmpdir, neff_name=neff_name)

    bu.compile_bir_kernel = patched
    b2j.compile_bir_kernel = patched


# --- kernel build ----------------------------------------------------------
def _build_nc():
    import concourse.bass as bass
    import concourse.mybir as mybir
    from concourse.tile import TileContext

    F32 = mybir.dt.float32
    AF = mybir.ActivationFunctionType
    OP = mybir.AluOpType
    AX = mybir.AxisListType

    nc = bass.Bass("TRN2", target_bir_lowering=False)

    resT = nc.dram_tensor("resT", [D, S], F32, kind="ExternalInput")
    cw = nc.dram_tensor("cw", [128, (D // 128) * C], F32, kind="ExternalInput")
    wih2r = nc.dram_tensor("wih2r", [128, 32], F32, kind="ExternalInput")
    wih2z = nc.dram_tensor("wih2z", [128, 32], F32, kind="ExternalInput")
    wih2n = nc.dram_tensor("wih2n", [128, 32], F32, kind="ExternalInput")
    wr = nc.dram_tensor("wr", [128, 32], F32, kind="ExternalInput")
    wz = nc.dram_tensor("wz", [128, 32], F32, kind="ExternalInput")
    wn = nc.dram_tensor("wn", [128, 32], F32, kind="ExternalInput")
    gmat = nc.dram_tensor("gmat", [128, 16], F32, kind="ExternalInput")
    smat = nc.dram_tensor("smat", [16, 128], F32, kind="ExternalInput")
    maskb = nc.dram_tensor("maskb", [128, NB], F32, kind="ExternalInput")
    selm = nc.dram_tensor("selm", [128, NB], F32, kind="ExternalInput")
    cb = nc.dram_tensor("cb", [C, 1], F32, kind="ExternalInput")
    brc = nc.dram_tensor("brc", [128, 1], F32, kind="ExternalInput")
    bzc = nc.dram_tensor("bzc", [128, 1], F32, kind="ExternalInput")
    bnc = nc.dram_tensor("bnc", [128, 1], F32, kind="ExternalInput")
    bhn = nc.dram_tensor("bhn", [128, 1], F32, kind="ExternalInput")
    out = nc.dram_tensor("out", [128, L], F32, kind="ExternalOutput")

    NDC = D // 128  # 16 d-chunks

    with TileContext(nc) as tc:
        with tc.tile_pool(name="const", bufs=1) as cst, \
             tc.tile_pool(name="persist", bufs=1) as per:
            cw_sb = cst.tile([128, NDC * C], F32, tag="cw")
            nc.sync.dma_start(cw_sb[:], cw.ap())
            wih2r_sb = cst.tile([128, 32], F32, tag="wih2r")
            nc.sync.dma_start(wih2r_sb[:], wih2r.ap())
            wih2z_sb = cst.tile([128, 32], F32, tag="wih2z")
            nc.sync.dma_start(wih2z_sb[:], wih2z.ap())
            wih2n_sb = cst.tile([128, 32], F32, tag="wih2n")
            nc.sync.dma_start(wih2n_sb[:], wih2n.ap())
            wr_sb = cst.tile([128, 32], F32, tag="wr")
            nc.sync.dma_start(wr_sb[:], wr.ap())
            wz_sb = cst.tile([128, 32], F32, tag="wz")
            nc.sync.dma_start(wz_sb[:], wz.ap())
            wn_sb = cst.tile([128, 32], F32, tag="wn")
            nc.sync.dma_start(wn_sb[:], wn.ap())
            g_sb = cst.tile([128, 16], F32, tag="g")
            nc.sync.dma_start(g_sb[:], gmat.ap())
            s_sb = cst.tile([16, 128], F32, tag="s")
            nc.sync.dma_start(s_sb[:], smat.ap())
            mb_sb = cst.tile([128, NB], F32, tag="mb")
            nc.sync.dma_start(mb_sb[:], maskb.ap())
            sel_sb = cst.tile([128, NB], F32, tag="sel")
            nc.sync.dma_start(sel_sb[:], selm.ap())
            cb_sb = cst.tile([C, 1], F32, tag="cb")
            nc.sync.dma_start(cb_sb[:], cb.ap())
            brc_sb = cst.tile([128, 1], F32, tag="brc")
            nc.sync.dma_start(brc_sb[:], brc.ap())
            bzc_sb = cst.tile([128, 1], F32, tag="bzc")
            nc.sync.dma_start(bzc_sb[:], bzc.ap())
            bnc_sb = cst.tile([128, 1], F32, tag="bnc")
            nc.sync.dma_start(bnc_sb[:], bnc.ap())
            bhn_sb = cst.tile([128, 1], F32, tag="bhn")
            nc.sync.dma_start(bhn_sb[:], bhn.ap())
            ones_sb = cst.tile([128, L], F32, tag="ones")
            nc.vector.memset(ones_sb[:], 1.0)

            # chunk-pair layout: rows 0-63 = even s-chunks, 64-127 = odd
            comp2 = per.tile([128, S // 2], F32, tag="comp2")
            xr_sb = per.tile([128, L], F32, tag="xr")
            xz_sb = per.tile([128, L], F32, tag="xz")
            xn_sb = per.tile([128, L], F32, tag="xn")

            # ---- Phase 1: compress matmul + gelu -------------------------
            with tc.tile_pool(name="resp", bufs=3) as resp, \
                 tc.tile_pool(name="cpsum", bufs=1, space="PSUM") as cpsum:
                ctiles = [cpsum.tile([C, L], F32, tag=f"c{sc}", name=f"c{sc}") for sc in range(NB)]
                for dc in range(NDC):
                    rt = resp.tile([128, S], F32, tag="res")
                    nc.sync.dma_start(rt[:], resT.ap()[dc * 128:(dc + 1) * 128, :])
                    for sc in range(NB):
                        nc.tensor.matmul(
                            ctiles[sc][:],
                            cw_sb[:, dc * C:(dc + 1) * C],
                            rt[:, sc * L:(sc + 1) * L],
                            start=(dc == 0), stop=(dc == NDC - 1),
                        )
                for sc in range(NB):
                    nc.scalar.activation(
                        comp2[64 * (sc % 2):64 * (sc % 2) + 64,
                              L * (sc // 2):L * (sc // 2) + L],
                        ctiles[sc][:], AF.Gelu, bias=cb_sb[:, 0:1],
                    )

            # ---- Phase 2: x-gates directly into blocked layout -----------
            # lhsT = blockdiag([wihT_g, wihT_g]) over the chunk-pair rows of
            # comp2 -> out [32, L] at partition 32j = blocks 2j (rows 0-15)
            # and 2j+1 (rows 16-31).
            with tc.tile_pool(name="xpsum", bufs=1, space="PSUM") as xpsum:
                for g, (wt2, dst, bias) in enumerate([
                    (wih2r_sb, xr_sb, brc_sb),
                    (wih2z_sb, xz_sb, bzc_sb),
                    (wih2n_sb, xn_sb, bnc_sb),
                ]):
                    ps = xpsum.tile([128, L], F32, tag=f"xg{g}", name=f"xg{g}")
                    for j in range(4):
                        nc.tensor.matmul(
                            ps[32 * j:32 * j + 32, :], wt2[:],
                            comp2[:, j * L:(j + 1) * L],
                            start=True, stop=True,
                            tile_position=(0, 32 * j),
                        )
                    nc.scalar.activation(
                        dst[:], ps[:], AF.Identity, bias=bias[:, 0:1],
                    )

            # ---- Phase 3: Picard sweeps ----------------------------------
            with tc.tile_pool(name="spsum", bufs=1, space="PSUM") as sps, \
                 tc.tile_pool(name="swp", bufs=1) as swp:
                rps = sps.tile([128, L], F32, tag="rps")
                zps = sps.tile([128, L], F32, tag="zps")
                wps = sps.tile([128, L], F32, tag="wps")
                ptps = sps.tile([16, NB], F32, tag="ptps")
                otps = sps.tile([16, NB], F32, tag="otps")
                ebps = sps.tile([128, NB], F32, tag="ebps")

                h0 = swp.tile([128, L], F32, tag="h0")
                h1 = swp.tile([128, L], F32, tag="h1")
                s_t = swp.tile([128, 2 * L], F32, tag="st")
                rz = swp.tile([128, 2 * L], F32, tag="rz")
                u_t = swp.tile([128, L], F32, tag="ut")
                v_t = swp.tile([128, L], F32, tag="vt")
                n_t = swp.tile([128, L], F32, tag="nt")
                c_t = swp.tile([128, L], F32, tag="ct")
                P_t = swp.tile([128, L], F32, tag="Pt")
                O_t = swp.tile([128, L], F32, tag="Ot")
                pm = swp.tile([128, NB], F32, tag="pm")
                om = swp.tile([128, NB], F32, tag="om")
                ot_sb = swp.tile([16, NB], F32, tag="otsb")
                hb_sb = swp.tile([16, NB], F32, tag="hbsb")
                eb_sb = swp.tile([128, NB], F32, tag="ebsb")
                hbc = swp.tile([128, 1], F32, tag="hbc")

                nc.vector.memset(h0[:], 0.0)
                nc.vector.memset(hbc[:], 0.0)
                h_cur, h_nxt = h0, h1
                for k in range(N_SWEEP):
                    # gates at time t use h_{t-1}: within-block shift by one
                    # column; column 0 uses the boundary column hbc (end of
                    # previous block, from the previous sweep's trajectory).
                    for wt, ps in ((wr_sb, rps), (wz_sb, zps), (wn_sb, wps)):
                        for j in range(4):
                            sl = slice(32 * j, 32 * j + 32)
                            nc.tensor.matmul(
                                ps[sl, 1:L], wt[sl, :], h_cur[sl, 0:L - 1],
                                start=True, stop=True,
                                tile_position=(32 * j, 32 * j),
                            )
                            nc.tensor.matmul(
                                ps[sl, 0:1], wt[sl, :], hbc[sl, 0:1],
                                start=True, stop=True,
                                tile_position=(32 * j, 32 * j),
                            )
                    nc.vector.tensor_tensor(s_t[:, 0:L], rps[:], xr_sb[:], OP.add)
                    nc.vector.tensor_tensor(s_t[:, L:2 * L], zps[:], xz_sb[:], OP.add)
                    nc.scalar.activation(rz[:], s_t[:], AF.Sigmoid)
                    nc.vector.scalar_tensor_tensor(
                        u_t[:], wps[:], bhn_sb[:, 0:1], rz[:, 0:L], OP.add, OP.mult)
                    nc.vector.tensor_tensor(v_t[:], u_t[:], xn_sb[:], OP.add)
                    nc.scalar.activation(n_t[:], v_t[:], AF.Tanh)
                    nc.vector.scalar_tensor_tensor(
                        c_t[:], rz[:, L:2 * L], 1.0, n_t[:], OP.subtract, OP.mult)
                    nc.vector.tensor_tensor_scan(
                        P_t[:], rz[:, L:2 * L], ones_sb[:], 1.0, OP.mult, OP.mult)
                    nc.vector.tensor_tensor_scan(
                        O_t[:], rz[:, L:2 * L], c_t[:], 0.0, OP.mult, OP.subtract)
                    # boundary chain across the 8 blocks
                    nc.vector.tensor_scalar(
                        pm[:], mb_sb[:], P_t[:, L - 1:L], None, OP.mult)
                    nc.vector.tensor_scalar(
                        om[:], mb_sb[:], O_t[:, L - 1:L], None, OP.mult)
                    nc.tensor.matmul(ptps[:], g_sb[:], pm[:], start=True, stop=True)
                    nc.tensor.matmul(otps[:], g_sb[:], om[:], start=True, stop=True)
                    nc.vector.tensor_copy(ot_sb[:], otps[:])
                    nc.vector.tensor_tensor_scan(
                        hb_sb[:], ptps[:], ot_sb[:], 0.0, OP.mult, OP.add)
                    nc.tensor.matmul(ebps[:], s_sb[:], hb_sb[:], start=True, stop=True)
                    nc.vector.tensor_tensor(eb_sb[:], ebps[:], sel_sb[:], OP.mult)
                    nc.vector.tensor_reduce(hbc[:], eb_sb[:], AX.X, OP.add)
                    nc.vector.scalar_tensor_tensor(
                        h_nxt[:], P_t[:], hbc[:, 0:1], O_t[:], OP.mult, OP.add)
                    h_cur, h_nxt = h_nxt, h_cur

                nc.sync.dma_start(out.ap(), h_cur[:])

    return nc


_NC_CACHE = None


def kernel(residual, compress_w, compress_b, w_ih, w_hh, b_ih, b_hh):
    global _NC_CACHE
    _install_bir_fix()
    from concourse.bass_utils import run_bass_kernel_spmd

    f32 = np.float32
    residual = np.ascontiguousarray(residual, dtype=f32)
    compress_w = np.asarray(compress_w, dtype=f32)
    compress_b = np.asarray(compress_b, dtype=f32)
    w_ih = np.asarray(w_ih, dtype=f32)
    w_hh = np.asarray(w_hh, dtype=f32)
    b_ih = np.asarray(b_ih, dtype=f32)
    b_hh = np.asarray(b_hh, dtype=f32)

    # host-side shared weight prep (layout only)
    cwT = np.ascontiguousarray(compress_w.T)                      # [D, C]
    cw_tiles = np.ascontiguousarray(
        cwT.reshape(D // 128, 128, C).transpose(1, 0, 2).reshape(128, -1))
    wihT = np.ascontiguousarray(w_ih.T)                           # [C, 3H]

    def wih2(g):   # [128, 32] blockdiag over chunk-pair rows
        m = np.zeros((128, 32), f32)
        m[0:C, 0:16] = wihT[:, 16 * g:16 * g + 16]
        m[C:2 * C, 16:32] = wihT[:, 16 * g:16 * g + 16]
        return m

    def blockdiag_tiled(w):   # w: [H, H] -> [128, 32]
        wT = w.T.astype(f32)
        b2 = np.zeros((32, 32), f32)
        b2[:16, :16] = wT
        b2[16:, 16:] = wT
        return np.ascontiguousarray(np.tile(b2, (4, 1)))

    wr_t = blockdiag_tiled(w_hh[:H])
    wz_t = blockdiag_tiled(w_hh[H:2 * H])
    wn_t = blockdiag_tiled(w_hh[2 * H:])

    g_np = np.zeros((128, 16), f32)
    for kk in range(128):
        g_np[kk, kk % 16] = 1.0
    s_np = np.ascontiguousarray(g_np.T)
    maskb_np = np.zeros((128, NB), f32)
    sel_np = np.zeros((128, NB), f32)
    for kk in range(128):
        maskb_np[kk, kk // 16] = 1.0
        if kk // 16 >= 1:
            sel_np[kk, kk // 16 - 1] = 1.0

    shared = {
        "cw": cw_tiles,
        "wih2r": wih2(0), "wih2z": wih2(1), "wih2n": wih2(2),
        "wr": wr_t, "wz": wz_t, "wn": wn_t,
        "gmat": g_np, "smat": s_np, "maskb": maskb_np, "selm": sel_np,
        "cb": compress_b.reshape(C, 1).astype(f32),
        "brc": np.ascontiguousarray(
            np.tile(b_ih[:H] + b_hh[:H], NB).reshape(128, 1)),
        "bzc": np.ascontiguousarray(
            np.tile(b_ih[H:2 * H] + b_hh[H:2 * H], NB).reshape(128, 1)),
        "bnc": np.ascontiguousarray(np.tile(b_ih[2 * H:], NB).reshape(128, 1)),
        "bhn": np.ascontiguousarray(np.tile(b_hh[2 * H:], NB).reshape(128, 1)),
    }

    in_maps = []
    for b in range(NCORES):
        m = dict(shared)
        m["resT"] = np.ascontiguousarray(residual[b].T)
        in_maps.append(m)

    if _NC_CACHE is None:
        _NC_CACHE = _build_nc()
    nc = _NC_CACHE

    res = run_bass_kernel_spmd(nc, in_maps, core_ids=list(range(NCORES)))
    if res.exec_time_ns is not None:
        print(f"HW exec time: {res.exec_time_ns} ns")

    out = np.zeros((B, S, H), f32)
    for b in range(NCORES):
        hb = res.results[b]["out"]                     # [128, L] blocked
        out[b] = hb.reshape(NB, H, L).transpose(0, 2, 1).reshape(S, H)
    return out


# revision 15
# speedup vs baseline: 1.1723x; 1.1723x over previous
"""Trainium2 Bass kernel for nn_AffectChannel (compress + GELU + 16-dim GRU scan).

Strategy (8 NeuronCores, data-parallel over batch, one batch element per core):
  Phase 1 (memory-bound): compressed = gelu(residual @ compress_w.T + b)
    - residual shard is pre-transposed on host -> fully coalesced DMA, fp32
      matmuls contract d on partitions, accumulate in PSUM.
  Phase 2: x_gates = compressed @ w_ih.T + biases, written in a "blocked"
    layout: partitions = 16 hidden lanes x 8 time-blocks (512 steps each).
  Phase 3: the sequential GRU scan is computed by Picard iteration: gates are
    evaluated from the previous trajectory estimate (fully parallel, 128-lane
    ops), then the diagonal blend recurrence h_t = z_t h_{t-1} + (1-z_t) n_t
    is solved EXACTLY with the DVE tensor_tensor_scan instruction (per-block
    prefix scans + an 8-block boundary chain via tiny PE gather/scatter
    matmuls).  ~24 sweeps converge to fp32 accuracy (contraction ~0.5/sweep).
"""
import json
import os

import numpy as np

B, S, D, C, H = 8, 4096, 2048, 64, 16
NB = 8           # time blocks
L = S // NB      # block length = 512
NCORES = 8
N_SWEEP = int(os.environ.get("AFFECT_N_SWEEP", "24"))


# --- walrus workaround: split multi-wait instructions ----------------------
def _split_multiwaits(d):
    n = 0
    uid = [0]
    for f in d.get("functions", []):
        for blk in f.get("blocks", []):
            out = []
            for ins in blk.get("instructions", []):
                si = ins.get("sync_info")
                waits = (si or {}).get("on_wait") or []
                if len(waits) > 1:
                    n += 1
                    for w in waits[:-1]:
                        uid[0] += 1
                        out.append({
                            "opcode": "EventSemaphore",
                            "name": f"{ins['name']}_wsplit{uid[0]}",
                            "engine": ins["engine"],
                            "ins": [], "outs": [],
                            "debug": ins.get("debug"),
                            "sync_info": {"on_wait": [w], "on_update": []},
                        })
                    si["on_wait"] = [waits[-1]]
                out.append(ins)
            blk["instructions"] = out
    return n


def _fix_bir_json(bir_json):
    if isinstance(bir_json, str):
        bir_json = bir_json.encode()
    d = json.loads(bir_json)
    if _split_multiwaits(d) == 0:
        return bir_json
    return json.dumps(d).encode()


_PATCHED = False


def _install_bir_fix():
    global _PATCHED
    if _PATCHED:
        return
    _PATCHED = True
    import concourse.bass_utils as bu
    import concourse.bass2jax as b2j

    orig = bu.compile_bir_kernel

    def patched(bir_json, tmpdir, neff_name="file.neff"):
        return orig(_fix_bir_json(bir_json), tmpdir, neff_name=neff_name)

    bu.compile_bir_kernel = patched
    b2j.compile_bir_kernel = patched


# --- kernel build ----------------------------------------------------------
def _build_nc():
    import concourse.bass as bass
    import concourse.mybir as mybir
    from concourse.tile import TileContext

    F32 = mybir.dt.float32
    AF = mybir.ActivationFunctionType
    OP = mybir.AluOpType
    AX = mybir.AxisListType

    nc = bass.Bass("TRN2", target_bir_lowering=False)

    resT = nc.dram_tensor("resT", [D, S], F32, kind="ExternalInput")
    cw = nc.dram_tensor("cw", [128, (D // 128) * C], F32, kind="ExternalInput")
    wih2r = nc.dram_tensor("wih2r", [128, 32], F32, kind="ExternalInput")
    wih2z = nc.dram_tensor("wih2z", [128, 32], F32, kind="ExternalInput")
    wih2n = nc.dram_tensor("wih2n", [128, 32], F32, kind="ExternalInput")
    wr = nc.dram_tensor("wr", [128, 32], F32, kind="ExternalInput")
    wz = nc.dram_tensor("wz", [128, 32], F32, kind="ExternalInput")
    wn = nc.dram_tensor("wn", [128, 32], F32, kind="ExternalInput")
    gmat = nc.dram_tensor("gmat", [128, 16], F32, kind="ExternalInput")
    smat = nc.dram_tensor("smat", [16, 128], F32, kind="ExternalInput")
    maskb = nc.dram_tensor("maskb", [128, NB], F32, kind="ExternalInput")
    selm = nc.dram_tensor("selm", [128, NB], F32, kind="ExternalInput")
    cb = nc.dram_tensor("cb", [C, 1], F32, kind="ExternalInput")
    brc = nc.dram_tensor("brc", [128, 1], F32, kind="ExternalInput")
    bzc = nc.dram_tensor("bzc", [128, 1], F32, kind="ExternalInput")
    bnc = nc.dram_tensor("bnc", [128, 1], F32, kind="ExternalInput")
    bhn = nc.dram_tensor("bhn", [128, 1], F32, kind="ExternalInput")
    out = nc.dram_tensor("out", [128, L], F32, kind="ExternalOutput")

    NDC = D // 128  # 16 d-chunks

    with TileContext(nc) as tc:
        with tc.tile_pool(name="const", bufs=1) as cst, \
             tc.tile_pool(name="persist", bufs=1) as per:
            cw_sb = cst.tile([128, NDC * C], F32, tag="cw")
            nc.sync.dma_start(cw_sb[:], cw.ap())
            wih2r_sb = cst.tile([128, 32], F32, tag="wih2r")
            nc.sync.dma_start(wih2r_sb[:], wih2r.ap())
            wih2z_sb = cst.tile([128, 32], F32, tag="wih2z")
            nc.sync.dma_start(wih2z_sb[:], wih2z.ap())
            wih2n_sb = cst.tile([128, 32], F32, tag="wih2n")
            nc.sync.dma_start(wih2n_sb[:], wih2n.ap())
            wr_sb = cst.tile([128, 32], F32, tag="wr")
            nc.sync.dma_start(wr_sb[:], wr.ap())
            wz_sb = cst.tile([128, 32], F32, tag="wz")
            nc.sync.dma_start(wz_sb[:], wz.ap())
            wn_sb = cst.tile([128, 32], F32, tag="wn")
            nc.sync.dma_start(wn_sb[:], wn.ap())
            g_sb = cst.tile([128, 16], F32, tag="g")
            nc.sync.dma_start(g_sb[:], gmat.ap())
            s_sb = cst.tile([16, 128], F32, tag="s")
            nc.sync.dma_start(s_sb[:], smat.ap())
            mb_sb = cst.tile([128, NB], F32, tag="mb")
            nc.sync.dma_start(mb_sb[:], maskb.ap())
            sel_sb = cst.tile([128, NB], F32, tag="sel")
            nc.sync.dma_start(sel_sb[:], selm.ap())
            cb_sb = cst.tile([C, 1], F32, tag="cb")
            nc.sync.dma_start(cb_sb[:], cb.ap())
            brc_sb = cst.tile([128, 1], F32, tag="brc")
            nc.sync.dma_start(brc_sb[:], brc.ap())
            bzc_sb = cst.tile([128, 1], F32, tag="bzc")
            nc.sync.dma_start(bzc_sb[:], bzc.ap())
            bnc_sb = cst.tile([128, 1], F32, tag="bnc")
            nc.sync.dma_start(bnc_sb[:], bnc.ap())
            bhn_sb = cst.tile([128, 1], F32, tag="bhn")
            nc.sync.dma_start(bhn_sb[:], bhn.ap())
            ones_sb = cst.tile([128, L], F32, tag="ones")
            nc.vector.memset(ones_sb[:], 1.0)

            # chunk-pair layout: rows 0-63 = even s-chunks, 64-127 = odd
            comp2 = per.tile([128, S // 2], F32, tag="comp2")
            xrz_sb = per.tile([128, 2 * L], F32, tag="xrz")
            xn_sb = per.tile([128, L], F32, tag="xn")

            # ---- Phase 1: compress matmul + gelu -------------------------
            with tc.tile_pool(name="resp", bufs=4) as resp, \
                 tc.tile_pool(name="cpsum", bufs=1, space="PSUM") as cpsum:
                ctiles = [cpsum.tile([C, L], F32, tag=f"c{sc}", name=f"c{sc}") for sc in range(NB)]
                for dc in range(NDC):
                    rt = resp.tile([128, S], F32, tag="res")
                    # split each 2MiB row-chunk across both HWDGE rings
                    nc.sync.dma_start(
                        rt[:, 0:S // 2],
                        resT.ap()[dc * 128:(dc + 1) * 128, 0:S // 2])
                    nc.scalar.dma_start(
                        rt[:, S // 2:S],
                        resT.ap()[dc * 128:(dc + 1) * 128, S // 2:S])
                    for sc in range(NB):
                        nc.tensor.matmul(
                            ctiles[sc][:],
                            cw_sb[:, dc * C:(dc + 1) * C],
                            rt[:, sc * L:(sc + 1) * L],
                            start=(dc == 0), stop=(dc == NDC - 1),
                        )
                for sc in range(NB):
                    nc.scalar.activation(
                        comp2[64 * (sc % 2):64 * (sc % 2) + 64,
                              L * (sc // 2):L * (sc // 2) + L],
                        ctiles[sc][:], AF.Gelu, bias=cb_sb[:, 0:1],
                    )

            # ---- Phase 2: x-gates directly into blocked layout -----------
            # lhsT = blockdiag([wihT_g, wihT_g]) over the chunk-pair rows of
            # comp2 -> out [32, L] at partition 32j = blocks 2j (rows 0-15)
            # and 2j+1 (rows 16-31).
            with tc.tile_pool(name="xpsum", bufs=1, space="PSUM") as xpsum:
                for g, (wt2, dst, bias) in enumerate([
                    (wih2r_sb, xrz_sb[:, 0:L], brc_sb),
                    (wih2z_sb, xrz_sb[:, L:2 * L], bzc_sb),
                    (wih2n_sb, xn_sb[:], bnc_sb),
                ]):
                    ps = xpsum.tile([128, L], F32, tag=f"xg{g}", name=f"xg{g}")
                    for j in range(4):
                        nc.tensor.matmul(
                            ps[32 * j:32 * j + 32, :], wt2[:],
                            comp2[:, j * L:(j + 1) * L],
                            start=True, stop=True,
                            tile_position=(0, 32 * j),
                        )
                    nc.scalar.activation(
                        dst, ps[:], AF.Identity, bias=bias[:, 0:1],
                    )

            # ---- Phase 3: Picard sweeps ----------------------------------
            # hs tiles hold the SHIFTED trajectory: hs[:, 0] = boundary
            # column (end of previous block = h_{t-1} for the block start),
            # hs[:, 1:L] = h[:, 0:L-1].  Gate matmuls then read hs directly.
            with tc.tile_pool(name="spsum", bufs=1, space="PSUM") as sps, \
                 tc.tile_pool(name="swp", bufs=1) as swp:
                rzps = sps.tile([128, 2 * L], F32, tag="rzps")
                wps = sps.tile([128, L], F32, tag="wps")
                ptps = sps.tile([16, NB], F32, tag="ptps")
                otps = sps.tile([16, NB], F32, tag="otps")
                ebps = sps.tile([128, NB], F32, tag="ebps")

                hs0 = swp.tile([128, L], F32, tag="hs0")
                hs1 = swp.tile([128, L], F32, tag="hs1")
                h_out = swp.tile([128, L], F32, tag="hout")
                s_t = swp.tile([128, 2 * L], F32, tag="st")
                rz = swp.tile([128, 2 * L], F32, tag="rz")
                u_t = swp.tile([128, L], F32, tag="ut")
                v_t = swp.tile([128, L], F32, tag="vt")
                n_t = swp.tile([128, L], F32, tag="nt")
                c_t = swp.tile([128, L], F32, tag="ct")
                P_t = swp.tile([128, L], F32, tag="Pt")
                O_t = swp.tile([128, L], F32, tag="Ot")
                pm = swp.tile([128, NB], F32, tag="pm")
                om = swp.tile([128, NB], F32, tag="om")
                ot_sb = swp.tile([16, NB], F32, tag="otsb")
                hb_sb = swp.tile([16, NB], F32, tag="hbsb")
                eb_sb = swp.tile([128, NB], F32, tag="ebsb")

                nc.vector.memset(hs0[:], 0.0)
                h_cur, h_nxt = hs0, hs1
                for k in range(N_SWEEP):
                    for wt, ps in ((wr_sb, rzps[:, 0:L]),
                                   (wz_sb, rzps[:, L:2 * L]),
                                   (wn_sb, wps[:])):
                        for j in range(4):
                            sl = slice(32 * j, 32 * j + 32)
                            nc.tensor.matmul(
                                ps[sl, :], wt[sl, :], h_cur[sl, :],
                                start=True, stop=True,
                                tile_position=(32 * j, 32 * j),
                            )
                    nc.vector.tensor_tensor(s_t[:], rzps[:], xrz_sb[:], OP.add)
                    nc.scalar.activation(rz[:], s_t[:], AF.Sigmoid)
                    nc.vector.scalar_tensor_tensor(
                        u_t[:], wps[:], bhn_sb[:, 0:1], rz[:, 0:L], OP.add, OP.mult)
                    nc.vector.tensor_tensor(v_t[:], u_t[:], xn_sb[:], OP.add)
                    nc.scalar.activation(n_t[:], v_t[:], AF.Tanh)
                    nc.vector.scalar_tensor_tensor(
                        c_t[:], rz[:, L:2 * L], 1.0, n_t[:], OP.subtract, OP.mult)
                    nc.vector.tensor_tensor_scan(
                        P_t[:], rz[:, L:2 * L], ones_sb[:], 1.0, OP.mult, OP.mult)
                    nc.vector.tensor_tensor_scan(
                        O_t[:], rz[:, L:2 * L], c_t[:], 0.0, OP.mult, OP.subtract)
                    # boundary chain across the 8 blocks
                    nc.vector.tensor_scalar(
                        pm[:], mb_sb[:], P_t[:, L - 1:L], None, OP.mult)
                    nc.vector.tensor_scalar(
                        om[:], mb_sb[:], O_t[:, L - 1:L], None, OP.mult)
                    nc.tensor.matmul(ptps[:], g_sb[:], pm[:], start=True, stop=True)
                    nc.tensor.matmul(otps[:], g_sb[:], om[:], start=True, stop=True)
                    nc.vector.tensor_copy(ot_sb[:], otps[:])
                    nc.vector.tensor_tensor_scan(
                        hb_sb[:], ptps[:], ot_sb[:], 0.0, OP.mult, OP.add)
                    nc.tensor.matmul(ebps[:], s_sb[:], hb_sb[:], start=True, stop=True)
                    nc.vector.tensor_tensor(eb_sb[:], ebps[:], sel_sb[:], OP.mult)
                    # boundary column -> hs_nxt[:, 0], shifted combine -> 1:L
                    nc.vector.tensor_reduce(h_nxt[:, 0:1], eb_sb[:], AX.X, OP.add)
                    nc.vector.scalar_tensor_tensor(
                        h_nxt[:, 1:L], P_t[:, 0:L - 1], h_nxt[:, 0:1],
                        O_t[:, 0:L - 1], OP.mult, OP.add)
                    if k == N_SWEEP - 1:
                        nc.vector.scalar_tensor_tensor(
                            h_out[:], P_t[:], h_nxt[:, 0:1], O_t[:],
                            OP.mult, OP.add)
                    h_cur, h_nxt = h_nxt, h_cur

                nc.sync.dma_start(out.ap(), h_out[:])

    return nc


_NC_CACHE = None


def kernel(residual, compress_w, compress_b, w_ih, w_hh, b_ih, b_hh):
    global _NC_CACHE
    _install_bir_fix()
    from concourse.bass_utils import run_bass_kernel_spmd

    f32 = np.float32
    residual = np.ascontiguousarray(residual, dtype=f32)
    compress_w = np.asarray(compress_w, dtype=f32)
    compress_b = np.asarray(compress_b, dtype=f32)
    w_ih = np.asarray(w_ih, dtype=f32)
    w_hh = np.asarray(w_hh, dtype=f32)
    b_ih = np.asarray(b_ih, dtype=f32)
    b_hh = np.asarray(b_hh, dtype=f32)

    # host-side shared weight prep (layout only)
    cwT = np.ascontiguousarray(compress_w.T)                      # [D, C]
    cw_tiles = np.ascontiguousarray(
        cwT.reshape(D // 128, 128, C).transpose(1, 0, 2).reshape(128, -1))
    wihT = np.ascontiguousarray(w_ih.T)                           # [C, 3H]

    def wih2(g):   # [128, 32] blockdiag over chunk-pair rows
        m = np.zeros((128, 32), f32)
        m[0:C, 0:16] = wihT[:, 16 * g:16 * g + 16]
        m[C:2 * C, 16:32] = wihT[:, 16 * g:16 * g + 16]
        return m

    def blockdiag_tiled(w):   # w: [H, H] -> [128, 32]
        wT = w.T.astype(f32)
        b2 = np.zeros((32, 32), f32)
        b2[:16, :16] = wT
        b2[16:, 16:] = wT
        return np.ascontiguousarray(np.tile(b2, (4, 1)))

    wr_t = blockdiag_tiled(w_hh[:H])
    wz_t = blockdiag_tiled(w_hh[H:2 * H])
    wn_t = blockdiag_tiled(w_hh[2 * H:])

    g_np = np.zeros((128, 16), f32)
    for kk in range(128):
        g_np[kk, kk % 16] = 1.0
    s_np = np.ascontiguousarray(g_np.T)
    maskb_np = np.zeros((128, NB), f32)
    sel_np = np.zeros((128, NB), f32)
    for kk in range(128):
        maskb_np[kk, kk // 16] = 1.0
        if kk // 16 >= 1:
            sel_np[kk, kk // 16 - 1] = 1.0

    shared = {
        "cw": cw_tiles,
        "wih2r": wih2(0), "wih2z": wih2(1), "wih2n": wih2(2),
        "wr": wr_t, "wz": wz_t, "wn": wn_t,
        "gmat": g_np, "smat": s_np, "maskb": maskb_np, "selm": sel_np,
        "cb": compress_b.reshape(C, 1).astype(f32),
        "brc": np.ascontiguousarray(
            np.tile(b_ih[:H] + b_hh[:H], NB).reshape(128, 1)),
        "bzc": np.ascontiguousarray(
            np.tile(b_ih[H:2 * H] + b_hh[H:2 * H], NB).reshape(128, 1)),
        "bnc": np.ascontiguousarray(np.tile(b_ih[2 * H:], NB).reshape(128, 1)),
        "bhn": np.ascontiguousarray(np.tile(b_hh[2 * H:], NB).reshape(128, 1)),
    }

    in_maps = []
    for b in range(NCORES):
        m = dict(shared)
        m["resT"] = np.ascontiguousarray(residual[b].T)
        in_maps.append(m)

    if _NC_CACHE is None:
        _NC_CACHE = _build_nc()
    nc = _NC_CACHE

    res = run_bass_kernel_spmd(nc, in_maps, core_ids=list(range(NCORES)))
    if res.exec_time_ns is not None:
        print(f"HW exec time: {res.exec_time_ns} ns")

    out = np.zeros((B, S, H), f32)
    for b in range(NCORES):
        hb = res.results[b]["out"]                     # [128, L] blocked
        out[b] = hb.reshape(NB, H, L).transpose(0, 2, 1).reshape(S, H)
    return out


# revision 16
# speedup vs baseline: 1.1879x; 1.0133x over previous
"""Trainium2 Bass kernel for nn_AffectChannel (compress + GELU + 16-dim GRU scan).

Strategy (8 NeuronCores, data-parallel over batch, one batch element per core):
  Phase 1 (memory-bound): compressed = gelu(residual @ compress_w.T + b)
    - residual shard is pre-transposed on host -> fully coalesced DMA, fp32
      matmuls contract d on partitions, accumulate in PSUM.
  Phase 2: x_gates = compressed @ w_ih.T + biases, written in a "blocked"
    layout: partitions = 16 hidden lanes x 8 time-blocks (512 steps each).
  Phase 3: the sequential GRU scan is computed by Picard iteration: gates are
    evaluated from the previous trajectory estimate (fully parallel, 128-lane
    ops), then the diagonal blend recurrence h_t = z_t h_{t-1} + (1-z_t) n_t
    is solved EXACTLY with the DVE tensor_tensor_scan instruction (per-block
    prefix scans + an 8-block boundary chain via tiny PE gather/scatter
    matmuls).  ~24 sweeps converge to fp32 accuracy (contraction ~0.5/sweep).
"""
import json
import os

import numpy as np

B, S, D, C, H = 8, 4096, 2048, 64, 16
NB = 8           # time blocks
L = S // NB      # block length = 512
NCORES = 8
N_SWEEP = int(os.environ.get("AFFECT_N_SWEEP", "24"))


# --- walrus workaround: split multi-wait instructions ----------------------
def _split_multiwaits(d):
    n = 0
    uid = [0]
    for f in d.get("functions", []):
        for blk in f.get("blocks", []):
            out = []
            for ins in blk.get("instructions", []):
                si = ins.get("sync_info")
                waits = (si or {}).get("on_wait") or []
                if len(waits) > 1:
                    n += 1
                    for w in waits[:-1]:
                        uid[0] += 1
                        out.append({
                            "opcode": "EventSemaphore",
                            "name": f"{ins['name']}_wsplit{uid[0]}",
                            "engine": ins["engine"],
                            "ins": [], "outs": [],
                            "debug": ins.get("debug"),
                            "sync_info": {"on_wait": [w], "on_update": []},
                        })
                    si["on_wait"] = [waits[-1]]
                out.append(ins)
            blk["instructions"] = out
    return n


def _fix_bir_json(bir_json):
    if isinstance(bir_json, str):
        bir_json = bir_json.encode()
    d = json.loads(bir_json)
    if _split_multiwaits(d) == 0:
        return bir_json
    return json.dumps(d).encode()


_PATCHED = False


def _install_bir_fix():
    global _PATCHED
    if _PATCHED:
        return
    _PATCHED = True
    import concourse.bass_utils as bu
    import concourse.bass2jax as b2j

    orig = bu.compile_bir_kernel

    def patched(bir_json, tmpdir, neff_name="file.neff"):
        return orig(_fix_bir_json(bir_json), tmpdir, neff_name=neff_name)

    bu.compile_bir_kernel = patched
    b2j.compile_bir_kernel = patched


# --- kernel build ----------------------------------------------------------
def _build_nc():
    import concourse.bass as bass
    import concourse.mybir as mybir
    from concourse.tile import TileContext

    F32 = mybir.dt.float32
    AF = mybir.ActivationFunctionType
    OP = mybir.AluOpType
    AX = mybir.AxisListType

    nc = bass.Bass("TRN2", target_bir_lowering=False)

    resT = nc.dram_tensor("resT", [D, S], F32, kind="ExternalInput")
    cw = nc.dram_tensor("cw", [128, (D // 128) * C], F32, kind="ExternalInput")
    wih2r = nc.dram_tensor("wih2r", [128, 32], F32, kind="ExternalInput")
    wih2z = nc.dram_tensor("wih2z", [128, 32], F32, kind="ExternalInput")
    wih2n = nc.dram_tensor("wih2n", [128, 32], F32, kind="ExternalInput")
    wr = nc.dram_tensor("wr", [128, 128], F32, kind="ExternalInput")
    wz = nc.dram_tensor("wz", [128, 128], F32, kind="ExternalInput")
    wn = nc.dram_tensor("wn", [128, 128], F32, kind="ExternalInput")
    gmat = nc.dram_tensor("gmat", [128, 16], F32, kind="ExternalInput")
    smat = nc.dram_tensor("smat", [16, 128], F32, kind="ExternalInput")
    maskb = nc.dram_tensor("maskb", [128, NB], F32, kind="ExternalInput")
    selm = nc.dram_tensor("selm", [128, NB], F32, kind="ExternalInput")
    cb = nc.dram_tensor("cb", [C, 1], F32, kind="ExternalInput")
    brc = nc.dram_tensor("brc", [128, 1], F32, kind="ExternalInput")
    bzc = nc.dram_tensor("bzc", [128, 1], F32, kind="ExternalInput")
    bnc = nc.dram_tensor("bnc", [128, 1], F32, kind="ExternalInput")
    bhn = nc.dram_tensor("bhn", [128, 1], F32, kind="ExternalInput")
    out = nc.dram_tensor("out", [128, L], F32, kind="ExternalOutput")

    NDC = D // 128  # 16 d-chunks

    with TileContext(nc) as tc:
        with tc.tile_pool(name="const", bufs=1) as cst, \
             tc.tile_pool(name="persist", bufs=1) as per:
            cw_sb = cst.tile([128, NDC * C], F32, tag="cw")
            nc.sync.dma_start(cw_sb[:], cw.ap())
            wih2r_sb = cst.tile([128, 32], F32, tag="wih2r")
            nc.sync.dma_start(wih2r_sb[:], wih2r.ap())
            wih2z_sb = cst.tile([128, 32], F32, tag="wih2z")
            nc.sync.dma_start(wih2z_sb[:], wih2z.ap())
            wih2n_sb = cst.tile([128, 32], F32, tag="wih2n")
            nc.sync.dma_start(wih2n_sb[:], wih2n.ap())
            wr_sb = cst.tile([128, 128], F32, tag="wr")
            nc.sync.dma_start(wr_sb[:], wr.ap())
            wz_sb = cst.tile([128, 128], F32, tag="wz")
            nc.sync.dma_start(wz_sb[:], wz.ap())
            wn_sb = cst.tile([128, 128], F32, tag="wn")
            nc.sync.dma_start(wn_sb[:], wn.ap())
            g_sb = cst.tile([128, 16], F32, tag="g")
            nc.sync.dma_start(g_sb[:], gmat.ap())
            s_sb = cst.tile([16, 128], F32, tag="s")
            nc.sync.dma_start(s_sb[:], smat.ap())
            mb_sb = cst.tile([128, NB], F32, tag="mb")
            nc.sync.dma_start(mb_sb[:], maskb.ap())
            sel_sb = cst.tile([128, NB], F32, tag="sel")
            nc.sync.dma_start(sel_sb[:], selm.ap())
            cb_sb = cst.tile([C, 1], F32, tag="cb")
            nc.sync.dma_start(cb_sb[:], cb.ap())
            brc_sb = cst.tile([128, 1], F32, tag="brc")
            nc.sync.dma_start(brc_sb[:], brc.ap())
            bzc_sb = cst.tile([128, 1], F32, tag="bzc")
            nc.sync.dma_start(bzc_sb[:], bzc.ap())
            bnc_sb = cst.tile([128, 1], F32, tag="bnc")
            nc.sync.dma_start(bnc_sb[:], bnc.ap())
            bhn_sb = cst.tile([128, 1], F32, tag="bhn")
            nc.sync.dma_start(bhn_sb[:], bhn.ap())
            ones_sb = cst.tile([128, L], F32, tag="ones")
            nc.vector.memset(ones_sb[:], 1.0)

            # chunk-pair layout: rows 0-63 = even s-chunks, 64-127 = odd
            comp2 = per.tile([128, S // 2], F32, tag="comp2")
            xrz_sb = per.tile([128, 2 * L], F32, tag="xrz")
            xn_sb = per.tile([128, L], F32, tag="xn")

            # ---- Phase 1: compress matmul + gelu -------------------------
            with tc.tile_pool(name="resp", bufs=4) as resp, \
                 tc.tile_pool(name="cpsum", bufs=1, space="PSUM") as cpsum:
                ctiles = [cpsum.tile([C, L], F32, tag=f"c{sc}", name=f"c{sc}") for sc in range(NB)]
                for dc in range(NDC):
                    rt = resp.tile([128, S], F32, tag="res")
                    # split each 2MiB row-chunk across both HWDGE rings
                    nc.sync.dma_start(
                        rt[:, 0:S // 2],
                        resT.ap()[dc * 128:(dc + 1) * 128, 0:S // 2])
                    nc.scalar.dma_start(
                        rt[:, S // 2:S],
                        resT.ap()[dc * 128:(dc + 1) * 128, S // 2:S])
                    for sc in range(NB):
                        nc.tensor.matmul(
                            ctiles[sc][:],
                            cw_sb[:, dc * C:(dc + 1) * C],
                            rt[:, sc * L:(sc + 1) * L],
                            start=(dc == 0), stop=(dc == NDC - 1),
                        )
                for sc in range(NB):
                    nc.scalar.activation(
                        comp2[64 * (sc % 2):64 * (sc % 2) + 64,
                              L * (sc // 2):L * (sc // 2) + L],
                        ctiles[sc][:], AF.Gelu, bias=cb_sb[:, 0:1],
                    )

            # ---- Phase 2: x-gates directly into blocked layout -----------
            # lhsT = blockdiag([wihT_g, wihT_g]) over the chunk-pair rows of
            # comp2 -> out [32, L] at partition 32j = blocks 2j (rows 0-15)
            # and 2j+1 (rows 16-31).
            with tc.tile_pool(name="xpsum", bufs=1, space="PSUM") as xpsum:
                for g, (wt2, dst, bias) in enumerate([
                    (wih2r_sb, xrz_sb[:, 0:L], brc_sb),
                    (wih2z_sb, xrz_sb[:, L:2 * L], bzc_sb),
                    (wih2n_sb, xn_sb[:], bnc_sb),
                ]):
                    ps = xpsum.tile([128, L], F32, tag=f"xg{g}", name=f"xg{g}")
                    for j in range(4):
                        nc.tensor.matmul(
                            ps[32 * j:32 * j + 32, :], wt2[:],
                            comp2[:, j * L:(j + 1) * L],
                            start=True, stop=True,
                            tile_position=(0, 32 * j),
                        )
                    nc.scalar.activation(
                        dst, ps[:], AF.Identity, bias=bias[:, 0:1],
                    )

            # ---- Phase 3: Picard sweeps ----------------------------------
            # hs tiles hold the SHIFTED trajectory: hs[:, 0] = boundary
            # column (end of previous block = h_{t-1} for the block start),
            # hs[:, 1:L] = h[:, 0:L-1].  Gate matmuls then read hs directly.
            with tc.tile_pool(name="spsum", bufs=1, space="PSUM") as sps, \
                 tc.tile_pool(name="swp", bufs=1) as swp:
                rzps = sps.tile([128, 2 * L], F32, tag="rzps")
                wps = sps.tile([128, L], F32, tag="wps")
                ptps = sps.tile([16, NB], F32, tag="ptps")
                otps = sps.tile([16, NB], F32, tag="otps")
                ebps = sps.tile([128, NB], F32, tag="ebps")

                hs0 = swp.tile([128, L], F32, tag="hs0")
                hs1 = swp.tile([128, L], F32, tag="hs1")
                h_out = swp.tile([128, L], F32, tag="hout")
                s_t = swp.tile([128, 2 * L], F32, tag="st")
                rz = swp.tile([128, 2 * L], F32, tag="rz")
                u_t = swp.tile([128, L], F32, tag="ut")
                v_t = swp.tile([128, L], F32, tag="vt")
                n_t = swp.tile([128, L], F32, tag="nt")
                c_t = swp.tile([128, L], F32, tag="ct")
                P_t = swp.tile([128, L], F32, tag="Pt")
                O_t = swp.tile([128, L], F32, tag="Ot")
                pm = swp.tile([128, NB], F32, tag="pm")
                om = swp.tile([128, NB], F32, tag="om")
                ot_sb = swp.tile([16, NB], F32, tag="otsb")
                hb_sb = swp.tile([16, NB], F32, tag="hbsb")
                eb_sb = swp.tile([128, NB], F32, tag="ebsb")

                nc.vector.memset(hs0[:], 0.0)
                h_cur, h_nxt = hs0, hs1
                for k in range(N_SWEEP):
                    for wt, ps in ((wr_sb, rzps[:, 0:L]),
                                   (wz_sb, rzps[:, L:2 * L]),
                                   (wn_sb, wps[:])):
                        nc.tensor.matmul(
                            ps[:], wt[:], h_cur[:], start=True, stop=True)
                    nc.vector.tensor_tensor(s_t[:], rzps[:], xrz_sb[:], OP.add)
                    nc.scalar.activation(rz[:], s_t[:], AF.Sigmoid)
                    nc.vector.scalar_tensor_tensor(
                        u_t[:], wps[:], bhn_sb[:, 0:1], rz[:, 0:L], OP.add, OP.mult)
                    nc.vector.tensor_tensor(v_t[:], u_t[:], xn_sb[:], OP.add)
                    nc.scalar.activation(n_t[:], v_t[:], AF.Tanh)
                    nc.vector.scalar_tensor_tensor(
                        c_t[:], rz[:, L:2 * L], 1.0, n_t[:], OP.subtract, OP.mult)
                    nc.vector.tensor_tensor_scan(
                        P_t[:], rz[:, L:2 * L], ones_sb[:], 1.0, OP.mult, OP.mult)
                    nc.vector.tensor_tensor_scan(
                        O_t[:], rz[:, L:2 * L], c_t[:], 0.0, OP.mult, OP.subtract)
                    # boundary chain across the 8 blocks
                    nc.vector.tensor_scalar(
                        pm[:], mb_sb[:], P_t[:, L - 1:L], None, OP.mult)
                    nc.vector.tensor_scalar(
                        om[:], mb_sb[:], O_t[:, L - 1:L], None, OP.mult)
                    nc.tensor.matmul(ptps[:], g_sb[:], pm[:], start=True, stop=True)
                    nc.tensor.matmul(otps[:], g_sb[:], om[:], start=True, stop=True)
                    nc.vector.tensor_copy(ot_sb[:], otps[:])
                    nc.vector.tensor_tensor_scan(
                        hb_sb[:], ptps[:], ot_sb[:], 0.0, OP.mult, OP.add)
                    nc.tensor.matmul(ebps[:], s_sb[:], hb_sb[:], start=True, stop=True)
                    nc.vector.tensor_tensor(eb_sb[:], ebps[:], sel_sb[:], OP.mult)
                    # boundary column -> hs_nxt[:, 0], shifted combine -> 1:L
                    nc.vector.tensor_reduce(h_nxt[:, 0:1], eb_sb[:], AX.X, OP.add)
                    nc.vector.scalar_tensor_tensor(
                        h_nxt[:, 1:L], P_t[:, 0:L - 1], h_nxt[:, 0:1],
                        O_t[:, 0:L - 1], OP.mult, OP.add)
                    if k == N_SWEEP - 1:
                        nc.vector.scalar_tensor_tensor(
                            h_out[:], P_t[:], h_nxt[:, 0:1], O_t[:],
                            OP.mult, OP.add)
                    h_cur, h_nxt = h_nxt, h_cur

                nc.sync.dma_start(out.ap(), h_out[:])

    return nc


_NC_CACHE = None


def kernel(residual, compress_w, compress_b, w_ih, w_hh, b_ih, b_hh):
    global _NC_CACHE
    _install_bir_fix()
    from concourse.bass_utils import run_bass_kernel_spmd

    f32 = np.float32
    residual = np.ascontiguousarray(residual, dtype=f32)
    compress_w = np.asarray(compress_w, dtype=f32)
    compress_b = np.asarray(compress_b, dtype=f32)
    w_ih = np.asarray(w_ih, dtype=f32)
    w_hh = np.asarray(w_hh, dtype=f32)
    b_ih = np.asarray(b_ih, dtype=f32)
    b_hh = np.asarray(b_hh, dtype=f32)

    # host-side shared weight prep (layout only)
    cwT = np.ascontiguousarray(compress_w.T)                      # [D, C]
    cw_tiles = np.ascontiguousarray(
        cwT.reshape(D // 128, 128, C).transpose(1, 0, 2).reshape(128, -1))
    wihT = np.ascontiguousarray(w_ih.T)                           # [C, 3H]

    def wih2(g):   # [128, 32] blockdiag over chunk-pair rows
        m = np.zeros((128, 32), f32)
        m[0:C, 0:16] = wihT[:, 16 * g:16 * g + 16]
        m[C:2 * C, 16:32] = wihT[:, 16 * g:16 * g + 16]
        return m

    def blockdiag_tiled(w):   # w: [H, H] -> [128, 128] (8 diagonal blocks)
        return np.ascontiguousarray(np.kron(np.eye(NB, dtype=f32), w.T.astype(f32)))

    wr_t = blockdiag_tiled(w_hh[:H])
    wz_t = blockdiag_tiled(w_hh[H:2 * H])
    wn_t = blockdiag_tiled(w_hh[2 * H:])

    g_np = np.zeros((128, 16), f32)
    for kk in range(128):
        g_np[kk, kk % 16] = 1.0
    s_np = np.ascontiguousarray(g_np.T)
    maskb_np = np.zeros((128, NB), f32)
    sel_np = np.zeros((128, NB), f32)
    for kk in range(128):
        maskb_np[kk, kk // 16] = 1.0
        if kk // 16 >= 1:
            sel_np[kk, kk // 16 - 1] = 1.0

    shared = {
        "cw": cw_tiles,
        "wih2r": wih2(0), "wih2z": wih2(1), "wih2n": wih2(2),
        "wr": wr_t, "wz": wz_t, "wn": wn_t,
        "gmat": g_np, "smat": s_np, "maskb": maskb_np, "selm": sel_np,
        "cb": compress_b.reshape(C, 1).astype(f32),
        "brc": np.ascontiguousarray(
            np.tile(b_ih[:H] + b_hh[:H], NB).reshape(128, 1)),
        "bzc": np.ascontiguousarray(
            np.tile(b_ih[H:2 * H] + b_hh[H:2 * H], NB).reshape(128, 1)),
        "bnc": np.ascontiguousarray(np.tile(b_ih[2 * H:], NB).reshape(128, 1)),
        "bhn": np.ascontiguousarray(np.tile(b_hh[2 * H:], NB).reshape(128, 1)),
    }

    in_maps = []
    for b in range(NCORES):
        m = dict(shared)
        m["resT"] = np.ascontiguousarray(residual[b].T)
        in_maps.append(m)

    if _NC_CACHE is None:
        _NC_CACHE = _build_nc()
    nc = _NC_CACHE

    res = run_bass_kernel_spmd(nc, in_maps, core_ids=list(range(NCORES)))
    if res.exec_time_ns is not None:
        print(f"HW exec time: {res.exec_time_ns} ns")

    out = np.zeros((B, S, H), f32)
    for b in range(NCORES):
        hb = res.results[b]["out"]                     # [128, L] blocked
        out[b] = hb.reshape(NB, H, L).transpose(0, 2, 1).reshape(S, H)
    return out


# revision 19
# speedup vs baseline: 1.1933x; 1.0045x over previous
"""Trainium2 Bass kernel for nn_AffectChannel (compress + GELU + 16-dim GRU scan).

Strategy (8 NeuronCores, data-parallel over batch, one batch element per core):
  Phase 1 (memory-bound): compressed = gelu(residual @ compress_w.T + b)
    - residual shard is pre-transposed on host -> fully coalesced DMA, fp32
      matmuls contract d on partitions, accumulate in PSUM.
  Phase 2: x_gates = compressed @ w_ih.T + biases, written in a "blocked"
    layout: partitions = 16 hidden lanes x 8 time-blocks (512 steps each).
  Phase 3: the sequential GRU scan is computed by Picard iteration: gates are
    evaluated from the previous trajectory estimate (fully parallel, 128-lane
    ops), then the diagonal blend recurrence h_t = z_t h_{t-1} + (1-z_t) n_t
    is solved EXACTLY with the DVE tensor_tensor_scan instruction (per-block
    prefix scans + an 8-block boundary chain via tiny PE gather/scatter
    matmuls).  ~24 sweeps converge to fp32 accuracy (contraction ~0.5/sweep).
"""
import json
import os

import numpy as np

B, S, D, C, H = 8, 4096, 2048, 64, 16
NB = 8           # time blocks
L = S // NB      # block length = 512
NCORES = 8
N_SWEEP = int(os.environ.get("AFFECT_N_SWEEP", "24"))


# --- walrus workaround: split multi-wait instructions ----------------------
def _split_multiwaits(d):
    n = 0
    uid = [0]
    for f in d.get("functions", []):
        for blk in f.get("blocks", []):
            out = []
            for ins in blk.get("instructions", []):
                si = ins.get("sync_info")
                waits = (si or {}).get("on_wait") or []
                if len(waits) > 1:
                    n += 1
                    for w in waits[:-1]:
                        uid[0] += 1
                        out.append({
                            "opcode": "EventSemaphore",
                            "name": f"{ins['name']}_wsplit{uid[0]}",
                            "engine": ins["engine"],
                            "ins": [], "outs": [],
                            "debug": ins.get("debug"),
                            "sync_info": {"on_wait": [w], "on_update": []},
                        })
                    si["on_wait"] = [waits[-1]]
                out.append(ins)
            blk["instructions"] = out
    return n


def _fix_bir_json(bir_json):
    if isinstance(bir_json, str):
        bir_json = bir_json.encode()
    d = json.loads(bir_json)
    if _split_multiwaits(d) == 0:
        return bir_json
    return json.dumps(d).encode()


_PATCHED = False


def _install_bir_fix():
    global _PATCHED
    if _PATCHED:
        return
    _PATCHED = True
    import concourse.bass_utils as bu
    import concourse.bass2jax as b2j

    orig = bu.compile_bir_kernel

    def patched(bir_json, tmpdir, neff_name="file.neff"):
        return orig(_fix_bir_json(bir_json), tmpdir, neff_name=neff_name)

    bu.compile_bir_kernel = patched
    b2j.compile_bir_kernel = patched


# --- kernel build ----------------------------------------------------------
def _build_nc():
    import concourse.bass as bass
    import concourse.mybir as mybir
    from concourse.tile import TileContext

    F32 = mybir.dt.float32
    AF = mybir.ActivationFunctionType
    OP = mybir.AluOpType
    AX = mybir.AxisListType

    nc = bass.Bass("TRN2", target_bir_lowering=False)

    resT = nc.dram_tensor("resT", [D, S], F32, kind="ExternalInput")
    cw = nc.dram_tensor("cw", [128, (D // 128) * C], F32, kind="ExternalInput")
    wih2r = nc.dram_tensor("wih2r", [128, 32], F32, kind="ExternalInput")
    wih2z = nc.dram_tensor("wih2z", [128, 32], F32, kind="ExternalInput")
    wih2n = nc.dram_tensor("wih2n", [128, 32], F32, kind="ExternalInput")
    wr = nc.dram_tensor("wr", [128, 128], F32, kind="ExternalInput")
    wz = nc.dram_tensor("wz", [128, 128], F32, kind="ExternalInput")
    wn = nc.dram_tensor("wn", [128, 128], F32, kind="ExternalInput")
    gmat = nc.dram_tensor("gmat", [128, 16], F32, kind="ExternalInput")
    smat = nc.dram_tensor("smat", [16, 128], F32, kind="ExternalInput")
    maskb = nc.dram_tensor("maskb", [128, NB], F32, kind="ExternalInput")
    selm = nc.dram_tensor("selm", [128, NB], F32, kind="ExternalInput")
    cb = nc.dram_tensor("cb", [C, 1], F32, kind="ExternalInput")
    brc = nc.dram_tensor("brc", [128, 1], F32, kind="ExternalInput")
    bzc = nc.dram_tensor("bzc", [128, 1], F32, kind="ExternalInput")
    bnc = nc.dram_tensor("bnc", [128, 1], F32, kind="ExternalInput")
    bhn = nc.dram_tensor("bhn", [128, 1], F32, kind="ExternalInput")
    out = nc.dram_tensor("out", [128, L], F32, kind="ExternalOutput")

    NDC = D // 128  # 16 d-chunks

    with TileContext(nc) as tc:
        with tc.tile_pool(name="const", bufs=1) as cst, \
             tc.tile_pool(name="persist", bufs=1) as per:
            cw_sb = cst.tile([128, NDC * C], F32, tag="cw")
            nc.sync.dma_start(cw_sb[:], cw.ap())
            wih2r_sb = cst.tile([128, 32], F32, tag="wih2r")
            nc.sync.dma_start(wih2r_sb[:], wih2r.ap())
            wih2z_sb = cst.tile([128, 32], F32, tag="wih2z")
            nc.sync.dma_start(wih2z_sb[:], wih2z.ap())
            wih2n_sb = cst.tile([128, 32], F32, tag="wih2n")
            nc.sync.dma_start(wih2n_sb[:], wih2n.ap())
            wr_sb = cst.tile([128, 128], F32, tag="wr")
            nc.sync.dma_start(wr_sb[:], wr.ap())
            wz_sb = cst.tile([128, 128], F32, tag="wz")
            nc.sync.dma_start(wz_sb[:], wz.ap())
            wn_sb = cst.tile([128, 128], F32, tag="wn")
            nc.sync.dma_start(wn_sb[:], wn.ap())
            g_sb = cst.tile([128, 16], F32, tag="g")
            nc.sync.dma_start(g_sb[:], gmat.ap())
            s_sb = cst.tile([16, 128], F32, tag="s")
            nc.sync.dma_start(s_sb[:], smat.ap())
            mb_sb = cst.tile([128, NB], F32, tag="mb")
            nc.sync.dma_start(mb_sb[:], maskb.ap())
            sel_sb = cst.tile([128, NB], F32, tag="sel")
            nc.sync.dma_start(sel_sb[:], selm.ap())
            cb_sb = cst.tile([C, 1], F32, tag="cb")
            nc.sync.dma_start(cb_sb[:], cb.ap())
            brc_sb = cst.tile([128, 1], F32, tag="brc")
            nc.sync.dma_start(brc_sb[:], brc.ap())
            bzc_sb = cst.tile([128, 1], F32, tag="bzc")
            nc.sync.dma_start(bzc_sb[:], bzc.ap())
            bnc_sb = cst.tile([128, 1], F32, tag="bnc")
            nc.sync.dma_start(bnc_sb[:], bnc.ap())
            bhn_sb = cst.tile([128, 1], F32, tag="bhn")
            nc.sync.dma_start(bhn_sb[:], bhn.ap())
            ones_sb = cst.tile([128, L], F32, tag="ones")
            nc.vector.memset(ones_sb[:], 1.0)

            # chunk-pair layout: rows 0-63 = even s-chunks, 64-127 = odd
            comp2 = per.tile([128, S // 2], F32, tag="comp2")
            xrz_sb = per.tile([128, 2 * L], F32, tag="xrz")
            xn_sb = per.tile([128, L], F32, tag="xn")

            # ---- Phase 1: compress matmul + gelu -------------------------
            with tc.tile_pool(name="resp", bufs=4) as resp, \
                 tc.tile_pool(name="cpsum", bufs=1, space="PSUM") as cpsum:
                ctiles = [cpsum.tile([C, L], F32, tag=f"c{sc}", name=f"c{sc}") for sc in range(NB)]
                for dc in range(NDC):
                    rt = resp.tile([128, S], F32, tag="res")
                    # split each 2MiB row-chunk across both HWDGE rings
                    nc.sync.dma_start(
                        rt[:, 0:S // 2],
                        resT.ap()[dc * 128:(dc + 1) * 128, 0:S // 2])
                    nc.scalar.dma_start(
                        rt[:, S // 2:S],
                        resT.ap()[dc * 128:(dc + 1) * 128, S // 2:S])
                    for sc in range(NB):
                        nc.tensor.matmul(
                            ctiles[sc][:],
                            cw_sb[:, dc * C:(dc + 1) * C],
                            rt[:, sc * L:(sc + 1) * L],
                            start=(dc == 0), stop=(dc == NDC - 1),
                        )
                for sc in range(NB):
                    nc.scalar.activation(
                        comp2[64 * (sc % 2):64 * (sc % 2) + 64,
                              L * (sc // 2):L * (sc // 2) + L],
                        ctiles[sc][:], AF.Gelu, bias=cb_sb[:, 0:1],
                    )

            # ---- Phase 2: x-gates directly into blocked layout -----------
            # lhsT = blockdiag([wihT_g, wihT_g]) over the chunk-pair rows of
            # comp2 -> out [32, L] at partition 32j = blocks 2j (rows 0-15)
            # and 2j+1 (rows 16-31).
            with tc.tile_pool(name="xpsum", bufs=1, space="PSUM") as xpsum:
                for g, (wt2, dst, bias) in enumerate([
                    (wih2r_sb, xrz_sb[:, 0:L], brc_sb),
                    (wih2z_sb, xrz_sb[:, L:2 * L], bzc_sb),
                    (wih2n_sb, xn_sb[:], bnc_sb),
                ]):
                    ps = xpsum.tile([128, L], F32, tag=f"xg{g}", name=f"xg{g}")
                    for j in range(4):
                        nc.tensor.matmul(
                            ps[32 * j:32 * j + 32, :], wt2[:],
                            comp2[:, j * L:(j + 1) * L],
                            start=True, stop=True,
                            tile_position=(0, 32 * j),
                        )
                    nc.scalar.activation(
                        dst, ps[:], AF.Identity, bias=bias[:, 0:1],
                    )

            # ---- Phase 3: Picard sweeps ----------------------------------
            # hs tiles hold the SHIFTED trajectory: hs[:, 0] = boundary
            # column (end of previous block = h_{t-1} for the block start),
            # hs[:, 1:L] = h[:, 0:L-1].  Gate matmuls then read hs directly.
            with tc.tile_pool(name="spsum", bufs=1, space="PSUM") as sps, \
                 tc.tile_pool(name="swp", bufs=1) as swp:
                rzps = sps.tile([128, 2 * L], F32, tag="rzps")
                wps = sps.tile([128, L], F32, tag="wps")
                ptps = sps.tile([16, NB], F32, tag="ptps")
                otps = sps.tile([16, NB], F32, tag="otps")
                ebps = sps.tile([128, NB], F32, tag="ebps")
                warm_ps = sps.tile([16, 16], F32, tag="warm")

                hs0 = swp.tile([128, L], F32, tag="hs0")
                hs1 = swp.tile([128, L], F32, tag="hs1")
                h_out = swp.tile([128, L], F32, tag="hout")
                s_t = swp.tile([128, 2 * L], F32, tag="st")
                rz = swp.tile([128, 2 * L], F32, tag="rz")
                u_t = swp.tile([128, L], F32, tag="ut")
                v_t = swp.tile([128, L], F32, tag="vt")
                n_t = swp.tile([128, L], F32, tag="nt")
                c_t = swp.tile([128, L], F32, tag="ct")
                P_t = swp.tile([128, L], F32, tag="Pt")
                O_t = swp.tile([128, L], F32, tag="Ot")
                pm = swp.tile([128, NB], F32, tag="pm")
                om = swp.tile([128, NB], F32, tag="om")
                ot_sb = swp.tile([16, NB], F32, tag="otsb")
                hb_sb = swp.tile([16, NB], F32, tag="hbsb")
                eb_sb = swp.tile([128, NB], F32, tag="ebsb")

                nc.vector.memset(hs0[:], 0.0)
                h_cur, h_nxt = hs0, hs1
                for k in range(N_SWEEP):
                    for wt, ps in ((wr_sb, rzps[:, 0:L]),
                                   (wz_sb, rzps[:, L:2 * L]),
                                   (wn_sb, wps[:])):
                        nc.tensor.matmul(
                            ps[:], wt[:], h_cur[:], start=True, stop=True)
                    nc.vector.tensor_tensor(s_t[:], rzps[:], xrz_sb[:], OP.add)
                    nc.scalar.activation(rz[:], s_t[:], AF.Sigmoid)
                    # tiny matmuls anchored mid-chain keep the PE HAM clock
                    # at 8/8 through the DVE/ACT stretch of each sweep
                    nc.tensor.matmul(warm_ps[:], rz[0:16, 0:16], rz[0:16, 0:16],
                                     start=True, stop=True)
                    nc.vector.scalar_tensor_tensor(
                        u_t[:], wps[:], bhn_sb[:, 0:1], rz[:, 0:L], OP.add, OP.mult)
                    nc.vector.tensor_tensor(v_t[:], u_t[:], xn_sb[:], OP.add)
                    nc.scalar.activation(n_t[:], v_t[:], AF.Tanh)
                    nc.tensor.matmul(warm_ps[:], n_t[0:16, 0:16], n_t[0:16, 0:16],
                                     start=True, stop=True)
                    nc.vector.scalar_tensor_tensor(
                        c_t[:], rz[:, L:2 * L], 1.0, n_t[:], OP.subtract, OP.mult)
                    nc.vector.tensor_tensor_scan(
                        P_t[:], rz[:, L:2 * L], ones_sb[:], 1.0, OP.mult, OP.mult)
                    nc.vector.tensor_tensor_scan(
                        O_t[:], rz[:, L:2 * L], c_t[:], 0.0, OP.mult, OP.subtract)
                    nc.tensor.matmul(warm_ps[:], c_t[0:16, 0:16], c_t[0:16, 0:16],
                                     start=True, stop=True)
                    # boundary chain across the 8 blocks
                    nc.vector.tensor_scalar(
                        pm[:], mb_sb[:], P_t[:, L - 1:L], None, OP.mult)
                    nc.vector.tensor_scalar(
                        om[:], mb_sb[:], O_t[:, L - 1:L], None, OP.mult)
                    nc.tensor.matmul(ptps[:], g_sb[:], pm[:], start=True, stop=True)
                    nc.tensor.matmul(otps[:], g_sb[:], om[:], start=True, stop=True)
                    nc.vector.tensor_copy(ot_sb[:], otps[:])
                    nc.vector.tensor_tensor_scan(
                        hb_sb[:], ptps[:], ot_sb[:], 0.0, OP.mult, OP.add)
                    nc.tensor.matmul(ebps[:], s_sb[:], hb_sb[:], start=True, stop=True)
                    nc.vector.tensor_tensor(eb_sb[:], ebps[:], sel_sb[:], OP.mult)
                    # boundary column -> hs_nxt[:, 0], shifted combine -> 1:L
                    nc.vector.tensor_reduce(h_nxt[:, 0:1], eb_sb[:], AX.X, OP.add)
                    nc.vector.scalar_tensor_tensor(
                        h_nxt[:, 1:L], P_t[:, 0:L - 1], h_nxt[:, 0:1],
                        O_t[:, 0:L - 1], OP.mult, OP.add)
                    if k == N_SWEEP - 1:
                        nc.vector.scalar_tensor_tensor(
                            h_out[:], P_t[:], h_nxt[:, 0:1], O_t[:],
                            OP.mult, OP.add)
                    h_cur, h_nxt = h_nxt, h_cur

                nc.sync.dma_start(out.ap(), h_out[:])

    return nc


_NC_CACHE = None


def kernel(residual, compress_w, compress_b, w_ih, w_hh, b_ih, b_hh):
    global _NC_CACHE
    _install_bir_fix()
    from concourse.bass_utils import run_bass_kernel_spmd

    f32 = np.float32
    residual = np.ascontiguousarray(residual, dtype=f32)
    compress_w = np.asarray(compress_w, dtype=f32)
    compress_b = np.asarray(compress_b, dtype=f32)
    w_ih = np.asarray(w_ih, dtype=f32)
    w_hh = np.asarray(w_hh, dtype=f32)
    b_ih = np.asarray(b_ih, dtype=f32)
    b_hh = np.asarray(b_hh, dtype=f32)

    # host-side shared weight prep (layout only)
    cwT = np.ascontiguousarray(compress_w.T)                      # [D, C]
    cw_tiles = np.ascontiguousarray(
        cwT.reshape(D // 128, 128, C).transpose(1, 0, 2).reshape(128, -1))
    wihT = np.ascontiguousarray(w_ih.T)                           # [C, 3H]

    def wih2(g):   # [128, 32] blockdiag over chunk-pair rows
        m = np.zeros((128, 32), f32)
        m[0:C, 0:16] = wihT[:, 16 * g:16 * g + 16]
        m[C:2 * C, 16:32] = wihT[:, 16 * g:16 * g + 16]
        return m

    def blockdiag_tiled(w):   # w: [H, H] -> [128, 128] (8 diagonal blocks)
        return np.ascontiguousarray(np.kron(np.eye(NB, dtype=f32), w.T.astype(f32)))

    wr_t = blockdiag_tiled(w_hh[:H])
    wz_t = blockdiag_tiled(w_hh[H:2 * H])
    wn_t = blockdiag_tiled(w_hh[2 * H:])

    g_np = np.zeros((128, 16), f32)
    for kk in range(128):
        g_np[kk, kk % 16] = 1.0
    s_np = np.ascontiguousarray(g_np.T)
    maskb_np = np.zeros((128, NB), f32)
    sel_np = np.zeros((128, NB), f32)
    for kk in range(128):
        maskb_np[kk, kk // 16] = 1.0
        if kk // 16 >= 1:
            sel_np[kk, kk // 16 - 1] = 1.0

    shared = {
        "cw": cw_tiles,
        "wih2r": wih2(0), "wih2z": wih2(1), "wih2n": wih2(2),
        "wr": wr_t, "wz": wz_t, "wn": wn_t,
        "gmat": g_np, "smat": s_np, "maskb": maskb_np, "selm": sel_np,
        "cb": compress_b.reshape(C, 1).astype(f32),
        "brc": np.ascontiguousarray(
            np.tile(b_ih[:H] + b_hh[:H], NB).reshape(128, 1)),
        "bzc": np.ascontiguousarray(
            np.tile(b_ih[H:2 * H] + b_hh[H:2 * H], NB).reshape(128, 1)),
        "bnc": np.ascontiguousarray(np.tile(b_ih[2 * H:], NB).reshape(128, 1)),
        "bhn": np.ascontiguousarray(np.tile(b_hh[2 * H:], NB).reshape(128, 1)),
    }

    in_maps = []
    for b in range(NCORES):
        m = dict(shared)
        m["resT"] = np.ascontiguousarray(residual[b].T)
        in_maps.append(m)

    if _NC_CACHE is None:
        _NC_CACHE = _build_nc()
    nc = _NC_CACHE

    res = run_bass_kernel_spmd(nc, in_maps, core_ids=list(range(NCORES)))
    if res.exec_time_ns is not None:
        print(f"HW exec time: {res.exec_time_ns} ns")

    out = np.zeros((B, S, H), f32)
    for b in range(NCORES):
        hb = res.results[b]["out"]                     # [128, L] blocked
        out[b] = hb.reshape(NB, H, L).transpose(0, 2, 1).reshape(S, H)
    return out


# revision 21
# speedup vs baseline: 1.2006x; 1.0061x over previous
"""Trainium2 Bass kernel for nn_AffectChannel (compress + GELU + 16-dim GRU scan).

Strategy (8 NeuronCores, data-parallel over batch, one batch element per core):
  Phase 1 (memory-bound): compressed = gelu(residual @ compress_w.T + b)
    - residual shard is pre-transposed on host -> fully coalesced DMA, fp32
      matmuls contract d on partitions, accumulate in PSUM.
  Phase 2: x_gates = compressed @ w_ih.T + biases, written in a "blocked"
    layout: partitions = 16 hidden lanes x 8 time-blocks (512 steps each).
  Phase 3: the sequential GRU scan is computed by Picard iteration: gates are
    evaluated from the previous trajectory estimate (fully parallel, 128-lane
    ops), then the diagonal blend recurrence h_t = z_t h_{t-1} + (1-z_t) n_t
    is solved EXACTLY with the DVE tensor_tensor_scan instruction (per-block
    prefix scans + an 8-block boundary chain via tiny PE gather/scatter
    matmuls).  ~24 sweeps converge to fp32 accuracy (contraction ~0.5/sweep).
"""
import json
import os

import numpy as np

B, S, D, C, H = 8, 4096, 2048, 64, 16
NB = 8           # time blocks
L = S // NB      # block length = 512
NCORES = 8
N_SWEEP = int(os.environ.get("AFFECT_N_SWEEP", "24"))


# --- walrus workaround: split multi-wait instructions ----------------------
def _split_multiwaits(d):
    n = 0
    uid = [0]
    for f in d.get("functions", []):
        for blk in f.get("blocks", []):
            out = []
            for ins in blk.get("instructions", []):
                si = ins.get("sync_info")
                waits = (si or {}).get("on_wait") or []
                if len(waits) > 1:
                    n += 1
                    for w in waits[:-1]:
                        uid[0] += 1
                        out.append({
                            "opcode": "EventSemaphore",
                            "name": f"{ins['name']}_wsplit{uid[0]}",
                            "engine": ins["engine"],
                            "ins": [], "outs": [],
                            "debug": ins.get("debug"),
                            "sync_info": {"on_wait": [w], "on_update": []},
                        })
                    si["on_wait"] = [waits[-1]]
                out.append(ins)
            blk["instructions"] = out
    return n


def _fix_bir_json(bir_json):
    if isinstance(bir_json, str):
        bir_json = bir_json.encode()
    d = json.loads(bir_json)
    if _split_multiwaits(d) == 0:
        return bir_json
    return json.dumps(d).encode()


_PATCHED = False


def _install_bir_fix():
    global _PATCHED
    if _PATCHED:
        return
    _PATCHED = True
    import concourse.bass_utils as bu
    import concourse.bass2jax as b2j

    orig = bu.compile_bir_kernel

    def patched(bir_json, tmpdir, neff_name="file.neff"):
        return orig(_fix_bir_json(bir_json), tmpdir, neff_name=neff_name)

    bu.compile_bir_kernel = patched
    b2j.compile_bir_kernel = patched


# --- kernel build ----------------------------------------------------------
def _build_nc():
    import concourse.bass as bass
    import concourse.mybir as mybir
    from concourse.tile import TileContext

    F32 = mybir.dt.float32
    AF = mybir.ActivationFunctionType
    OP = mybir.AluOpType
    AX = mybir.AxisListType

    nc = bass.Bass("TRN2", target_bir_lowering=False)

    resT = nc.dram_tensor("resT", [D, S], F32, kind="ExternalInput")
    cw = nc.dram_tensor("cw", [128, (D // 128) * C], F32, kind="ExternalInput")
    wih2r = nc.dram_tensor("wih2r", [128, 32], F32, kind="ExternalInput")
    wih2z = nc.dram_tensor("wih2z", [128, 32], F32, kind="ExternalInput")
    wih2n = nc.dram_tensor("wih2n", [128, 32], F32, kind="ExternalInput")
    wr = nc.dram_tensor("wr", [128, 128], F32, kind="ExternalInput")
    wz = nc.dram_tensor("wz", [128, 128], F32, kind="ExternalInput")
    wn = nc.dram_tensor("wn", [128, 128], F32, kind="ExternalInput")
    gmat = nc.dram_tensor("gmat", [128, 16], F32, kind="ExternalInput")
    smat = nc.dram_tensor("smat", [16, 128], F32, kind="ExternalInput")
    maskb = nc.dram_tensor("maskb", [128, NB], F32, kind="ExternalInput")
    selm = nc.dram_tensor("selm", [128, NB], F32, kind="ExternalInput")
    cb2 = nc.dram_tensor("cb2", [128, 1], F32, kind="ExternalInput")
    brc = nc.dram_tensor("brc", [128, 1], F32, kind="ExternalInput")
    bzc = nc.dram_tensor("bzc", [128, 1], F32, kind="ExternalInput")
    bnc = nc.dram_tensor("bnc", [128, 1], F32, kind="ExternalInput")
    bhn = nc.dram_tensor("bhn", [128, 1], F32, kind="ExternalInput")
    out = nc.dram_tensor("out", [128, L], F32, kind="ExternalOutput")

    NDC = D // 128  # 16 d-chunks

    with TileContext(nc) as tc:
        with tc.tile_pool(name="const", bufs=1) as cst, \
             tc.tile_pool(name="persist", bufs=1) as per:
            cw_sb = cst.tile([128, NDC * C], F32, tag="cw")
            nc.sync.dma_start(cw_sb[:], cw.ap())
            wih2r_sb = cst.tile([128, 32], F32, tag="wih2r")
            nc.sync.dma_start(wih2r_sb[:], wih2r.ap())
            wih2z_sb = cst.tile([128, 32], F32, tag="wih2z")
            nc.sync.dma_start(wih2z_sb[:], wih2z.ap())
            wih2n_sb = cst.tile([128, 32], F32, tag="wih2n")
            nc.sync.dma_start(wih2n_sb[:], wih2n.ap())
            wr_sb = cst.tile([128, 128], F32, tag="wr")
            nc.sync.dma_start(wr_sb[:], wr.ap())
            wz_sb = cst.tile([128, 128], F32, tag="wz")
            nc.sync.dma_start(wz_sb[:], wz.ap())
            wn_sb = cst.tile([128, 128], F32, tag="wn")
            nc.sync.dma_start(wn_sb[:], wn.ap())
            g_sb = cst.tile([128, 16], F32, tag="g")
            nc.sync.dma_start(g_sb[:], gmat.ap())
            s_sb = cst.tile([16, 128], F32, tag="s")
            nc.sync.dma_start(s_sb[:], smat.ap())
            mb_sb = cst.tile([128, NB], F32, tag="mb")
            nc.sync.dma_start(mb_sb[:], maskb.ap())
            sel_sb = cst.tile([128, NB], F32, tag="sel")
            nc.sync.dma_start(sel_sb[:], selm.ap())
            cb2_sb = cst.tile([128, 1], F32, tag="cb2")
            nc.sync.dma_start(cb2_sb[:], cb2.ap())
            brc_sb = cst.tile([128, 1], F32, tag="brc")
            nc.sync.dma_start(brc_sb[:], brc.ap())
            bzc_sb = cst.tile([128, 1], F32, tag="bzc")
            nc.sync.dma_start(bzc_sb[:], bzc.ap())
            bnc_sb = cst.tile([128, 1], F32, tag="bnc")
            nc.sync.dma_start(bnc_sb[:], bnc.ap())
            bhn_sb = cst.tile([128, 1], F32, tag="bhn")
            nc.sync.dma_start(bhn_sb[:], bhn.ap())
            ones_sb = cst.tile([128, L], F32, tag="ones")
            nc.vector.memset(ones_sb[:], 1.0)

            # chunk-pair layout: rows 0-63 = even s-chunks, 64-127 = odd
            comp2 = per.tile([128, S // 2], F32, tag="comp2")
            xrz_sb = per.tile([128, 2 * L], F32, tag="xrz")
            xn_sb = per.tile([128, L], F32, tag="xn")

            # ---- Phase 1: compress matmul + gelu -------------------------
            # s-chunk pairs run CONCURRENTLY on the PE via column tiling:
            # even chunk -> array col strips 0-63, odd chunk -> 64-127.
            # The [128, L] psum pair tile is already in comp2's layout.
            with tc.tile_pool(name="resp", bufs=4) as resp, \
                 tc.tile_pool(name="cpsum", bufs=1, space="PSUM") as cpsum:
                ctiles = [cpsum.tile([128, L], F32, tag=f"c{p}", name=f"c{p}") for p in range(4)]
                for dc in range(NDC):
                    rt = resp.tile([128, S], F32, tag="res")
                    # split each 2MiB row-chunk across both HWDGE rings
                    nc.sync.dma_start(
                        rt[:, 0:S // 2],
                        resT.ap()[dc * 128:(dc + 1) * 128, 0:S // 2])
                    nc.scalar.dma_start(
                        rt[:, S // 2:S],
                        resT.ap()[dc * 128:(dc + 1) * 128, S // 2:S])
                    for p in range(4):
                        nc.tensor.matmul(
                            ctiles[p][0:64, :],
                            cw_sb[:, dc * C:(dc + 1) * C],
                            rt[:, (2 * p) * L:(2 * p + 1) * L],
                            start=(dc == 0), stop=(dc == NDC - 1),
                            tile_position=(0, 0),
                        )
                        nc.tensor.matmul(
                            ctiles[p][64:128, :],
                            cw_sb[:, dc * C:(dc + 1) * C],
                            rt[:, (2 * p + 1) * L:(2 * p + 2) * L],
                            start=(dc == 0), stop=(dc == NDC - 1),
                            tile_position=(0, 64),
                        )
                for p in range(4):
                    nc.scalar.activation(
                        comp2[:, L * p:L * (p + 1)],
                        ctiles[p][:], AF.Gelu, bias=cb2_sb[:, 0:1],
                    )

            # ---- Phase 2: x-gates directly into blocked layout -----------
            # lhsT = blockdiag([wihT_g, wihT_g]) over the chunk-pair rows of
            # comp2 -> out [32, L] at partition 32j = blocks 2j (rows 0-15)
            # and 2j+1 (rows 16-31).
            with tc.tile_pool(name="xpsum", bufs=1, space="PSUM") as xpsum:
                for g, (wt2, dst, bias) in enumerate([
                    (wih2r_sb, xrz_sb[:, 0:L], brc_sb),
                    (wih2z_sb, xrz_sb[:, L:2 * L], bzc_sb),
                    (wih2n_sb, xn_sb[:], bnc_sb),
                ]):
                    ps = xpsum.tile([128, L], F32, tag=f"xg{g}", name=f"xg{g}")
                    for j in range(4):
                        nc.tensor.matmul(
                            ps[32 * j:32 * j + 32, :], wt2[:],
                            comp2[:, j * L:(j + 1) * L],
                            start=True, stop=True,
                            tile_position=(0, 32 * j),
                        )
                    nc.scalar.activation(
                        dst, ps[:], AF.Identity, bias=bias[:, 0:1],
                    )

            # ---- Phase 3: Picard sweeps ----------------------------------
            # hs tiles hold the SHIFTED trajectory: hs[:, 0] = boundary
            # column (end of previous block = h_{t-1} for the block start),
            # hs[:, 1:L] = h[:, 0:L-1].  Gate matmuls then read hs directly.
            with tc.tile_pool(name="spsum", bufs=1, space="PSUM") as sps, \
                 tc.tile_pool(name="swp", bufs=1) as swp:
                rzps = sps.tile([128, 2 * L], F32, tag="rzps")
                wps = sps.tile([128, L], F32, tag="wps")
                ptps = sps.tile([16, NB], F32, tag="ptps")
                otps = sps.tile([16, NB], F32, tag="otps")
                ebps = sps.tile([128, NB], F32, tag="ebps")
                warm_ps = sps.tile([32, L], F32, tag="warm")

                hs0 = swp.tile([128, L], F32, tag="hs0")
                hs1 = swp.tile([128, L], F32, tag="hs1")
                h_out = swp.tile([128, L], F32, tag="hout")
                s_t = swp.tile([128, 2 * L], F32, tag="st")
                rz = swp.tile([128, 2 * L], F32, tag="rz")
                u_t = swp.tile([128, L], F32, tag="ut")
                v_t = swp.tile([128, L], F32, tag="vt")
                n_t = swp.tile([128, L], F32, tag="nt")
                c_t = swp.tile([128, L], F32, tag="ct")
                P_t = swp.tile([128, L], F32, tag="Pt")
                O_t = swp.tile([128, L], F32, tag="Ot")
                pm = swp.tile([128, NB], F32, tag="pm")
                om = swp.tile([128, NB], F32, tag="om")
                ot_sb = swp.tile([16, NB], F32, tag="otsb")
                hb_sb = swp.tile([16, NB], F32, tag="hbsb")
                eb_sb = swp.tile([128, NB], F32, tag="ebsb")

                nc.vector.memset(hs0[:], 0.0)
                h_cur, h_nxt = hs0, hs1
                for k in range(N_SWEEP):
                    for wt, ps in ((wr_sb, rzps[:, 0:L]),
                                   (wz_sb, rzps[:, L:2 * L]),
                                   (wn_sb, wps[:])):
                        nc.tensor.matmul(
                            ps[:], wt[:], h_cur[:], start=True, stop=True)
                    nc.vector.tensor_tensor(s_t[:], rzps[:], xrz_sb[:], OP.add)
                    nc.scalar.activation(rz[:], s_t[:], AF.Sigmoid)
                    # chain-anchored matmuls keep the PE HAM clock at 8/8
                    # through the DVE/ACT stretch of each sweep
                    nc.tensor.matmul(warm_ps[:], rz[0:32, 0:32], rz[0:32, 0:L],
                                     start=True, stop=True)
                    nc.vector.scalar_tensor_tensor(
                        u_t[:], wps[:], bhn_sb[:, 0:1], rz[:, 0:L], OP.add, OP.mult)
                    nc.vector.tensor_tensor(v_t[:], u_t[:], xn_sb[:], OP.add)
                    nc.scalar.activation(n_t[:], v_t[:], AF.Tanh)
                    nc.tensor.matmul(warm_ps[:], v_t[0:32, 0:32], v_t[0:32, 0:L],
                                     start=True, stop=True)
                    nc.vector.scalar_tensor_tensor(
                        c_t[:], rz[:, L:2 * L], 1.0, n_t[:], OP.subtract, OP.mult)
                    nc.vector.tensor_tensor_scan(
                        O_t[:], rz[:, L:2 * L], c_t[:], 0.0, OP.mult, OP.subtract)
                    nc.vector.tensor_tensor_scan(
                        P_t[:], rz[:, L:2 * L], ones_sb[:], 1.0, OP.mult, OP.mult)
                    nc.tensor.matmul(warm_ps[:], c_t[0:32, 0:32], c_t[0:32, 0:L],
                                     start=True, stop=True)
                    # boundary chain across the 8 blocks
                    nc.vector.tensor_scalar(
                        pm[:], mb_sb[:], P_t[:, L - 1:L], None, OP.mult)
                    nc.vector.tensor_scalar(
                        om[:], mb_sb[:], O_t[:, L - 1:L], None, OP.mult)
                    nc.tensor.matmul(ptps[:], g_sb[:], pm[:], start=True, stop=True)
                    nc.tensor.matmul(otps[:], g_sb[:], om[:], start=True, stop=True)
                    nc.vector.tensor_copy(ot_sb[:], otps[:])
                    nc.vector.tensor_tensor_scan(
                        hb_sb[:], ptps[:], ot_sb[:], 0.0, OP.mult, OP.add)
                    nc.tensor.matmul(ebps[:], s_sb[:], hb_sb[:], start=True, stop=True)
                    nc.vector.tensor_tensor(eb_sb[:], ebps[:], sel_sb[:], OP.mult)
                    # boundary column -> hs_nxt[:, 0], shifted combine -> 1:L
                    nc.vector.tensor_reduce(h_nxt[:, 0:1], eb_sb[:], AX.X, OP.add)
                    nc.vector.scalar_tensor_tensor(
                        h_nxt[:, 1:L], P_t[:, 0:L - 1], h_nxt[:, 0:1],
                        O_t[:, 0:L - 1], OP.mult, OP.add)
                    if k == N_SWEEP - 1:
                        nc.vector.scalar_tensor_tensor(
                            h_out[:], P_t[:], h_nxt[:, 0:1], O_t[:],
                            OP.mult, OP.add)
                    h_cur, h_nxt = h_nxt, h_cur

                nc.sync.dma_start(out.ap(), h_out[:])

    return nc


_NC_CACHE = None


def kernel(residual, compress_w, compress_b, w_ih, w_hh, b_ih, b_hh):
    global _NC_CACHE
    _install_bir_fix()
    from concourse.bass_utils import run_bass_kernel_spmd

    f32 = np.float32
    residual = np.ascontiguousarray(residual, dtype=f32)
    compress_w = np.asarray(compress_w, dtype=f32)
    compress_b = np.asarray(compress_b, dtype=f32)
    w_ih = np.asarray(w_ih, dtype=f32)
    w_hh = np.asarray(w_hh, dtype=f32)
    b_ih = np.asarray(b_ih, dtype=f32)
    b_hh = np.asarray(b_hh, dtype=f32)

    # host-side shared weight prep (layout only)
    cwT = np.ascontiguousarray(compress_w.T)                      # [D, C]
    cw_tiles = np.ascontiguousarray(
        cwT.reshape(D // 128, 128, C).transpose(1, 0, 2).reshape(128, -1))
    wihT = np.ascontiguousarray(w_ih.T)                           # [C, 3H]

    def wih2(g):   # [128, 32] blockdiag over chunk-pair rows
        m = np.zeros((128, 32), f32)
        m[0:C, 0:16] = wihT[:, 16 * g:16 * g + 16]
        m[C:2 * C, 16:32] = wihT[:, 16 * g:16 * g + 16]
        return m

    def blockdiag_tiled(w):   # w: [H, H] -> [128, 128] (8 diagonal blocks)
        return np.ascontiguousarray(np.kron(np.eye(NB, dtype=f32), w.T.astype(f32)))

    wr_t = blockdiag_tiled(w_hh[:H])
    wz_t = blockdiag_tiled(w_hh[H:2 * H])
    wn_t = blockdiag_tiled(w_hh[2 * H:])

    g_np = np.zeros((128, 16), f32)
    for kk in range(128):
        g_np[kk, kk % 16] = 1.0
    s_np = np.ascontiguousarray(g_np.T)
    maskb_np = np.zeros((128, NB), f32)
    sel_np = np.zeros((128, NB), f32)
    for kk in range(128):
        maskb_np[kk, kk // 16] = 1.0
        if kk // 16 >= 1:
            sel_np[kk, kk // 16 - 1] = 1.0

    shared = {
        "cw": cw_tiles,
        "wih2r": wih2(0), "wih2z": wih2(1), "wih2n": wih2(2),
        "wr": wr_t, "wz": wz_t, "wn": wn_t,
        "gmat": g_np, "smat": s_np, "maskb": maskb_np, "selm": sel_np,
        "cb2": np.ascontiguousarray(np.tile(compress_b, 2).reshape(128, 1)),
        "brc": np.ascontiguousarray(
            np.tile(b_ih[:H] + b_hh[:H], NB).reshape(128, 1)),
        "bzc": np.ascontiguousarray(
            np.tile(b_ih[H:2 * H] + b_hh[H:2 * H], NB).reshape(128, 1)),
        "bnc": np.ascontiguousarray(np.tile(b_ih[2 * H:], NB).reshape(128, 1)),
        "bhn": np.ascontiguousarray(np.tile(b_hh[2 * H:], NB).reshape(128, 1)),
    }

    in_maps = []
    for b in range(NCORES):
        m = dict(shared)
        m["resT"] = np.ascontiguousarray(residual[b].T)
        in_maps.append(m)

    if _NC_CACHE is None:
        _NC_CACHE = _build_nc()
    nc = _NC_CACHE

    res = run_bass_kernel_spmd(nc, in_maps, core_ids=list(range(NCORES)))
    if res.exec_time_ns is not None:
        print(f"HW exec time: {res.exec_time_ns} ns")

    out = np.zeros((B, S, H), f32)
    for b in range(NCORES):
        hb = res.results[b]["out"]                     # [128, L] blocked
        out[b] = hb.reshape(NB, H, L).transpose(0, 2, 1).reshape(S, H)
    return out


# revision 23
# speedup vs baseline: 1.2510x; 1.0419x over previous
"""Trainium2 Bass kernel for nn_AffectChannel (compress + GELU + 16-dim GRU scan).

Strategy (8 NeuronCores, data-parallel over batch, one batch element per core):
  Phase 1 (memory-bound): compressed = gelu(residual @ compress_w.T + b)
    - residual shard is pre-transposed on host -> fully coalesced DMA, fp32
      matmuls contract d on partitions, accumulate in PSUM.
  Phase 2: x_gates = compressed @ w_ih.T + biases, written in a "blocked"
    layout: partitions = 16 hidden lanes x 8 time-blocks (512 steps each).
  Phase 3: the sequential GRU scan is computed by Picard iteration: gates are
    evaluated from the previous trajectory estimate (fully parallel, 128-lane
    ops), then the diagonal blend recurrence h_t = z_t h_{t-1} + (1-z_t) n_t
    is solved EXACTLY with the DVE tensor_tensor_scan instruction (per-block
    prefix scans + an 8-block boundary chain via tiny PE gather/scatter
    matmuls).  ~24 sweeps converge to fp32 accuracy (contraction ~0.5/sweep).
"""
import json
import os

import numpy as np

B, S, D, C, H = 8, 4096, 2048, 64, 16
NB = 8           # time blocks
L = S // NB      # block length = 512
NCORES = 8
N_SWEEP = int(os.environ.get("AFFECT_N_SWEEP", "22"))


# --- walrus workaround: split multi-wait instructions ----------------------
def _split_multiwaits(d):
    n = 0
    uid = [0]
    for f in d.get("functions", []):
        for blk in f.get("blocks", []):
            out = []
            for ins in blk.get("instructions", []):
                si = ins.get("sync_info")
                waits = (si or {}).get("on_wait") or []
                if len(waits) > 1:
                    n += 1
                    for w in waits[:-1]:
                        uid[0] += 1
                        out.append({
                            "opcode": "EventSemaphore",
                            "name": f"{ins['name']}_wsplit{uid[0]}",
                            "engine": ins["engine"],
                            "ins": [], "outs": [],
                            "debug": ins.get("debug"),
                            "sync_info": {"on_wait": [w], "on_update": []},
                        })
                    si["on_wait"] = [waits[-1]]
                out.append(ins)
            blk["instructions"] = out
    return n


def _fix_bir_json(bir_json):
    if isinstance(bir_json, str):
        bir_json = bir_json.encode()
    d = json.loads(bir_json)
    if _split_multiwaits(d) == 0:
        return bir_json
    return json.dumps(d).encode()


_PATCHED = False


def _install_bir_fix():
    global _PATCHED
    if _PATCHED:
        return
    _PATCHED = True
    import concourse.bass_utils as bu
    import concourse.bass2jax as b2j

    orig = bu.compile_bir_kernel

    def patched(bir_json, tmpdir, neff_name="file.neff"):
        return orig(_fix_bir_json(bir_json), tmpdir, neff_name=neff_name)

    bu.compile_bir_kernel = patched
    b2j.compile_bir_kernel = patched


# --- kernel build ----------------------------------------------------------
def _build_nc():
    import concourse.bass as bass
    import concourse.mybir as mybir
    from concourse.tile import TileContext

    F32 = mybir.dt.float32
    AF = mybir.ActivationFunctionType
    OP = mybir.AluOpType
    AX = mybir.AxisListType

    nc = bass.Bass("TRN2", target_bir_lowering=False)

    resT = nc.dram_tensor("resT", [D, S], F32, kind="ExternalInput")
    # all constants packed into one tensor: [cw(1024) wr wz wn(128*3) wih2*(32*3)
    #  gmat(16) smat_pad(128) maskb selm(8+8) cols(1*5)] = 1669 cols
    consts = nc.dram_tensor("consts", [128, 1669], F32, kind="ExternalInput")
    out = nc.dram_tensor("out", [128, L], F32, kind="ExternalOutput")

    NDC = D // 128  # 16 d-chunks

    with TileContext(nc) as tc:
        with tc.tile_pool(name="const", bufs=1) as cst, \
             tc.tile_pool(name="persist", bufs=1) as per:
            call = cst.tile([128, 1669], F32, tag="call")
            nc.sync.dma_start(call[:], consts.ap())
            cw_sb = call[:, 0:1024]
            wr_sb = call[:, 1024:1152]
            wz_sb = call[:, 1152:1280]
            wn_sb = call[:, 1280:1408]
            wih2r_sb = call[:, 1408:1440]
            wih2z_sb = call[:, 1440:1472]
            wih2n_sb = call[:, 1472:1504]
            g_sb = call[:, 1504:1520]
            s_sb = call[0:16, 1520:1648]
            mb_sb = call[:, 1648:1656]
            sel_sb = call[:, 1656:1664]
            cb2_sb = call[:, 1664:1665]
            brc_sb = call[:, 1665:1666]
            bzc_sb = call[:, 1666:1667]
            bnc_sb = call[:, 1667:1668]
            bhn_sb = call[:, 1668:1669]
            ones_sb = cst.tile([128, L], F32, tag="ones")
            nc.vector.memset(ones_sb, 1.0)

            # chunk-pair layout: rows 0-63 = even s-chunks, 64-127 = odd
            comp2 = per.tile([128, S // 2], F32, tag="comp2")
            xrz_sb = per.tile([128, 2 * L], F32, tag="xrz")
            xn_sb = per.tile([128, L], F32, tag="xn")

            # ---- Phase 1: compress matmul + gelu -------------------------
            # s-chunk pairs run CONCURRENTLY on the PE via column tiling:
            # even chunk -> array col strips 0-63, odd chunk -> 64-127.
            # The [128, L] psum pair tile is already in comp2's layout.
            with tc.tile_pool(name="resp", bufs=4) as resp, \
                 tc.tile_pool(name="cpsum", bufs=1, space="PSUM") as cpsum:
                ctiles = [cpsum.tile([128, L], F32, tag=f"c{p}", name=f"c{p}") for p in range(4)]
                for dc in range(NDC):
                    rt = resp.tile([128, S], F32, tag="res")
                    # split each 2MiB row-chunk across both HWDGE rings
                    nc.sync.dma_start(
                        rt[:, 0:S // 2],
                        resT.ap()[dc * 128:(dc + 1) * 128, 0:S // 2])
                    nc.scalar.dma_start(
                        rt[:, S // 2:S],
                        resT.ap()[dc * 128:(dc + 1) * 128, S // 2:S])
                    for p in range(4):
                        nc.tensor.matmul(
                            ctiles[p][0:64, :],
                            cw_sb[:, dc * C:(dc + 1) * C],
                            rt[:, (2 * p) * L:(2 * p + 1) * L],
                            start=(dc == 0), stop=(dc == NDC - 1),
                            tile_position=(0, 0),
                        )
                        nc.tensor.matmul(
                            ctiles[p][64:128, :],
                            cw_sb[:, dc * C:(dc + 1) * C],
                            rt[:, (2 * p + 1) * L:(2 * p + 2) * L],
                            start=(dc == 0), stop=(dc == NDC - 1),
                            tile_position=(0, 64),
                        )
                for p in range(4):
                    nc.scalar.activation(
                        comp2[:, L * p:L * (p + 1)],
                        ctiles[p][:], AF.Gelu, bias=cb2_sb,
                    )

            # ---- Phase 2: x-gates directly into blocked layout -----------
            # lhsT = blockdiag([wihT_g, wihT_g]) over the chunk-pair rows of
            # comp2 -> out [32, L] at partition 32j = blocks 2j (rows 0-15)
            # and 2j+1 (rows 16-31).
            with tc.tile_pool(name="xpsum", bufs=1, space="PSUM") as xpsum:
                for g, (wt2, dst, bias) in enumerate([
                    (wih2r_sb, xrz_sb[:, 0:L], brc_sb),
                    (wih2z_sb, xrz_sb[:, L:2 * L], bzc_sb),
                    (wih2n_sb, xn_sb[:], bnc_sb),
                ]):
                    ps = xpsum.tile([128, L], F32, tag=f"xg{g}", name=f"xg{g}")
                    for j in range(4):
                        nc.tensor.matmul(
                            ps[32 * j:32 * j + 32, :], wt2[:],
                            comp2[:, j * L:(j + 1) * L],
                            start=True, stop=True,
                            tile_position=(0, 32 * j),
                        )
                    nc.scalar.activation(
                        dst, ps[:], AF.Identity, bias=bias[:, 0:1],
                    )

            # ---- Phase 3: Picard sweeps ----------------------------------
            # hs tiles hold the SHIFTED trajectory: hs[:, 0] = boundary
            # column (end of previous block = h_{t-1} for the block start),
            # hs[:, 1:L] = h[:, 0:L-1].  Gate matmuls then read hs directly.
            with tc.tile_pool(name="spsum", bufs=1, space="PSUM") as sps, \
                 tc.tile_pool(name="swp", bufs=1) as swp:
                rzps = sps.tile([128, 2 * L], F32, tag="rzps")
                wps = sps.tile([128, L], F32, tag="wps")
                ptps = sps.tile([16, NB], F32, tag="ptps")
                otps = sps.tile([16, NB], F32, tag="otps")
                ebps = sps.tile([128, NB], F32, tag="ebps")
                warm_ps = sps.tile([32, L], F32, tag="warm")

                hs0 = swp.tile([128, L], F32, tag="hs0")
                hs1 = swp.tile([128, L], F32, tag="hs1")
                h_out = swp.tile([128, L], F32, tag="hout")
                s_t = swp.tile([128, 2 * L], F32, tag="st")
                rz = swp.tile([128, 2 * L], F32, tag="rz")
                u_t = swp.tile([128, L], F32, tag="ut")
                v_t = swp.tile([128, L], F32, tag="vt")
                n_t = swp.tile([128, L], F32, tag="nt")
                c_t = swp.tile([128, L], F32, tag="ct")
                P_t = swp.tile([128, L], F32, tag="Pt")
                O_t = swp.tile([128, L], F32, tag="Ot")
                pm = swp.tile([128, NB], F32, tag="pm")
                om = swp.tile([128, NB], F32, tag="om")
                ot_sb = swp.tile([16, NB], F32, tag="otsb")
                hb_sb = swp.tile([16, NB], F32, tag="hbsb")
                eb_sb = swp.tile([128, NB], F32, tag="ebsb")

                nc.vector.memset(hs0[:], 0.0)
                h_cur, h_nxt = hs0, hs1
                for k in range(N_SWEEP):
                    for wt, ps in ((wr_sb, rzps[:, 0:L]),
                                   (wz_sb, rzps[:, L:2 * L]),
                                   (wn_sb, wps[:])):
                        nc.tensor.matmul(
                            ps[:], wt[:], h_cur[:], start=True, stop=True)
                    nc.vector.tensor_tensor(s_t[:], rzps[:], xrz_sb[:], OP.add)
                    nc.scalar.activation(rz[:], s_t[:], AF.Sigmoid)
                    # chain-anchored matmuls keep the PE HAM clock at 8/8
                    # through the DVE/ACT stretch of each sweep
                    nc.tensor.matmul(warm_ps[:], rz[0:32, 0:32], rz[0:32, 0:L],
                                     start=True, stop=True)
                    nc.vector.scalar_tensor_tensor(
                        u_t[:], wps[:], bhn_sb, rz[:, 0:L], OP.add, OP.mult)
                    nc.vector.tensor_tensor(v_t[:], u_t[:], xn_sb[:], OP.add)
                    nc.scalar.activation(n_t[:], v_t[:], AF.Tanh)
                    nc.tensor.matmul(warm_ps[:], v_t[0:32, 0:32], v_t[0:32, 0:L],
                                     start=True, stop=True)
                    nc.vector.scalar_tensor_tensor(
                        c_t[:], rz[:, L:2 * L], 1.0, n_t[:], OP.subtract, OP.mult)
                    nc.vector.tensor_tensor_scan(
                        O_t[:], rz[:, L:2 * L], c_t[:], 0.0, OP.mult, OP.subtract)
                    nc.vector.tensor_tensor_scan(
                        P_t[:], rz[:, L:2 * L], ones_sb, 1.0, OP.mult, OP.mult)
                    nc.tensor.matmul(warm_ps[:], c_t[0:32, 0:32], c_t[0:32, 0:L],
                                     start=True, stop=True)
                    # boundary chain across the 8 blocks
                    nc.vector.tensor_scalar(
                        pm[:], mb_sb, P_t[:, L - 1:L], None, OP.mult)
                    nc.vector.tensor_scalar(
                        om[:], mb_sb, O_t[:, L - 1:L], None, OP.mult)
                    nc.tensor.matmul(ptps[:], g_sb, pm[:], start=True, stop=True)
                    nc.tensor.matmul(otps[:], g_sb, om[:], start=True, stop=True)
                    nc.vector.tensor_copy(ot_sb[:], otps[:])
                    nc.vector.tensor_tensor_scan(
                        hb_sb[:], ptps[:], ot_sb[:], 0.0, OP.mult, OP.add)
                    nc.tensor.matmul(ebps[:], s_sb, hb_sb[:], start=True, stop=True)
                    nc.vector.tensor_tensor(eb_sb[:], ebps[:], sel_sb, OP.mult)
                    nc.tensor.matmul(warm_ps[0:8, :], eb_sb[0:32, 0:8], rz[0:32, 0:L],
                                     start=True, stop=True)
                    # boundary column -> hs_nxt[:, 0], shifted combine -> 1:L
                    nc.vector.tensor_reduce(h_nxt[:, 0:1], eb_sb[:], AX.X, OP.add)
                    nc.vector.scalar_tensor_tensor(
                        h_nxt[:, 1:L], P_t[:, 0:L - 1], h_nxt[:, 0:1],
                        O_t[:, 0:L - 1], OP.mult, OP.add)
                    if k == N_SWEEP - 1:
                        nc.vector.scalar_tensor_tensor(
                            h_out[:], P_t[:], h_nxt[:, 0:1], O_t[:],
                            OP.mult, OP.add)
                    h_cur, h_nxt = h_nxt, h_cur

                nc.sync.dma_start(out.ap(), h_out[:])

    return nc


_NC_CACHE = None


def kernel(residual, compress_w, compress_b, w_ih, w_hh, b_ih, b_hh):
    global _NC_CACHE
    _install_bir_fix()
    from concourse.bass_utils import run_bass_kernel_spmd

    f32 = np.float32
    residual = np.ascontiguousarray(residual, dtype=f32)
    compress_w = np.asarray(compress_w, dtype=f32)
    compress_b = np.asarray(compress_b, dtype=f32)
    w_ih = np.asarray(w_ih, dtype=f32)
    w_hh = np.asarray(w_hh, dtype=f32)
    b_ih = np.asarray(b_ih, dtype=f32)
    b_hh = np.asarray(b_hh, dtype=f32)

    # host-side shared weight prep (layout only)
    cwT = np.ascontiguousarray(compress_w.T)                      # [D, C]
    cw_tiles = np.ascontiguousarray(
        cwT.reshape(D // 128, 128, C).transpose(1, 0, 2).reshape(128, -1))
    wihT = np.ascontiguousarray(w_ih.T)                           # [C, 3H]

    def wih2(g):   # [128, 32] blockdiag over chunk-pair rows
        m = np.zeros((128, 32), f32)
        m[0:C, 0:16] = wihT[:, 16 * g:16 * g + 16]
        m[C:2 * C, 16:32] = wihT[:, 16 * g:16 * g + 16]
        return m

    def blockdiag_tiled(w):   # w: [H, H] -> [128, 128] (8 diagonal blocks)
        return np.ascontiguousarray(np.kron(np.eye(NB, dtype=f32), w.T.astype(f32)))

    wr_t = blockdiag_tiled(w_hh[:H])
    wz_t = blockdiag_tiled(w_hh[H:2 * H])
    wn_t = blockdiag_tiled(w_hh[2 * H:])

    g_np = np.zeros((128, 16), f32)
    for kk in range(128):
        g_np[kk, kk % 16] = 1.0
    s_np = np.ascontiguousarray(g_np.T)
    maskb_np = np.zeros((128, NB), f32)
    sel_np = np.zeros((128, NB), f32)
    for kk in range(128):
        maskb_np[kk, kk // 16] = 1.0
        if kk // 16 >= 1:
            sel_np[kk, kk // 16 - 1] = 1.0

    smat_pad = np.zeros((128, 128), f32)
    smat_pad[0:16, :] = s_np
    consts_np = np.concatenate([
        cw_tiles, wr_t, wz_t, wn_t, wih2(0), wih2(1), wih2(2), g_np, smat_pad,
        maskb_np, sel_np,
        np.tile(compress_b, 2).reshape(128, 1),
        np.tile(b_ih[:H] + b_hh[:H], NB).reshape(128, 1),
        np.tile(b_ih[H:2 * H] + b_hh[H:2 * H], NB).reshape(128, 1),
        np.tile(b_ih[2 * H:], NB).reshape(128, 1),
        np.tile(b_hh[2 * H:], NB).reshape(128, 1),
    ], axis=1).astype(f32)
    shared = {"consts": np.ascontiguousarray(consts_np)}

    in_maps = []
    for b in range(NCORES):
        m = dict(shared)
        m["resT"] = np.ascontiguousarray(residual[b].T)
        in_maps.append(m)

    if _NC_CACHE is None:
        _NC_CACHE = _build_nc()
    nc = _NC_CACHE

    res = run_bass_kernel_spmd(nc, in_maps, core_ids=list(range(NCORES)))
    if res.exec_time_ns is not None:
        print(f"HW exec time: {res.exec_time_ns} ns")

    out = np.zeros((B, S, H), f32)
    for b in range(NCORES):
        hb = res.results[b]["out"]                     # [128, L] blocked
        out[b] = hb.reshape(NB, H, L).transpose(0, 2, 1).reshape(S, H)
    return out


# revision 28
# speedup vs baseline: 1.3942x; 1.1145x over previous
"""Trainium2 Bass kernel for nn_AffectChannel (compress + GELU + 16-dim GRU scan).

Strategy (8 NeuronCores, data-parallel over batch, one batch element per core):
  Phase 1 (memory-bound): compressed = gelu(residual @ compress_w.T + b)
    - residual shard is pre-transposed on host -> fully coalesced DMA, fp32
      matmuls contract d on partitions, accumulate in PSUM.
  Phase 2: x_gates = compressed @ w_ih.T + biases, written in a "blocked"
    layout: partitions = 16 hidden lanes x 8 time-blocks (512 steps each).
  Phase 3: the sequential GRU scan is computed by Picard iteration: gates are
    evaluated from the previous trajectory estimate (fully parallel, 128-lane
    ops), then the diagonal blend recurrence h_t = z_t h_{t-1} + (1-z_t) n_t
    is solved EXACTLY with the DVE tensor_tensor_scan instruction (per-block
    prefix scans + an 8-block boundary chain via tiny PE gather/scatter
    matmuls).  ~24 sweeps converge to fp32 accuracy (contraction ~0.5/sweep).
"""
import json
import os

import numpy as np

B, S, D, C, H = 8, 4096, 2048, 64, 16
NB = 8           # time blocks
L = S // NB      # block length = 512
NCORES = 8
N_SWEEP = int(os.environ.get("AFFECT_N_SWEEP", "23"))
N_BF = int(os.environ.get("AFFECT_N_BF", "13"))


# --- walrus workaround: split multi-wait instructions ----------------------
def _split_multiwaits(d):
    n = 0
    uid = [0]
    for f in d.get("functions", []):
        for blk in f.get("blocks", []):
            out = []
            for ins in blk.get("instructions", []):
                si = ins.get("sync_info")
                waits = (si or {}).get("on_wait") or []
                if len(waits) > 1:
                    n += 1
                    for w in waits[:-1]:
                        uid[0] += 1
                        out.append({
                            "opcode": "EventSemaphore",
                            "name": f"{ins['name']}_wsplit{uid[0]}",
                            "engine": ins["engine"],
                            "ins": [], "outs": [],
                            "debug": ins.get("debug"),
                            "sync_info": {"on_wait": [w], "on_update": []},
                        })
                    si["on_wait"] = [waits[-1]]
                out.append(ins)
            blk["instructions"] = out
    return n


def _fix_bir_json(bir_json):
    if isinstance(bir_json, str):
        bir_json = bir_json.encode()
    d = json.loads(bir_json)
    if _split_multiwaits(d) == 0:
        return bir_json
    return json.dumps(d).encode()


_PATCHED = False


def _install_bir_fix():
    global _PATCHED
    if _PATCHED:
        return
    _PATCHED = True
    import concourse.bass_utils as bu
    import concourse.bass2jax as b2j

    orig = bu.compile_bir_kernel

    def patched(bir_json, tmpdir, neff_name="file.neff"):
        return orig(_fix_bir_json(bir_json), tmpdir, neff_name=neff_name)

    bu.compile_bir_kernel = patched
    b2j.compile_bir_kernel = patched


# --- kernel build ----------------------------------------------------------
def _build_nc():
    import concourse.bass as bass
    import concourse.mybir as mybir
    from concourse.tile import TileContext

    F32 = mybir.dt.float32
    BF16 = mybir.dt.bfloat16
    AF = mybir.ActivationFunctionType
    OP = mybir.AluOpType
    AX = mybir.AxisListType

    nc = bass.Bass("TRN2", target_bir_lowering=False)

    resT = nc.dram_tensor("resT", [D, S], F32, kind="ExternalInput")
    # all constants packed into one tensor: [cw(1024) wr wz wn(128*3) wih2*(32*3)
    #  gmat(16) smat_pad(128) maskb selm(8+8) cols(1*5)] = 1669 cols
    consts = nc.dram_tensor("consts", [128, 1669], F32, kind="ExternalInput")
    wbf = nc.dram_tensor("wbf", [128, 384], mybir.dt.bfloat16, kind="ExternalInput")
    out = nc.dram_tensor("out", [128, L], F32, kind="ExternalOutput")

    NDC = D // 128  # 16 d-chunks

    with TileContext(nc) as tc:
        with tc.tile_pool(name="const", bufs=1) as cst, \
             tc.tile_pool(name="persist", bufs=1) as per:
            call = cst.tile([128, 1669], F32, tag="call")
            nc.sync.dma_start(call[:], consts.ap())
            cw_sb = call[:, 0:1024]
            wr_sb = call[:, 1024:1152]
            wz_sb = call[:, 1152:1280]
            wn_sb = call[:, 1280:1408]
            wih2r_sb = call[:, 1408:1440]
            wih2z_sb = call[:, 1440:1472]
            wih2n_sb = call[:, 1472:1504]
            g_sb = call[:, 1504:1520]
            s_sb = call[0:16, 1520:1648]
            mb_sb = call[:, 1648:1656]
            sel_sb = call[:, 1656:1664]
            cb2_sb = call[:, 1664:1665]
            brc_sb = call[:, 1665:1666]
            bzc_sb = call[:, 1666:1667]
            bnc_sb = call[:, 1667:1668]
            bhn_sb = call[:, 1668:1669]
            wbf_sb = cst.tile([128, 384], BF16, tag="wbf")
            nc.sync.dma_start(wbf_sb[:], wbf.ap())
            wrb_sb = wbf_sb[:, 0:128]
            wzb_sb = wbf_sb[:, 128:256]
            wnb_sb = wbf_sb[:, 256:384]
            ones_sb = cst.tile([128, L], F32, tag="ones")
            nc.vector.memset(ones_sb, 1.0)

            # chunk-pair layout: rows 0-63 = even s-chunks, 64-127 = odd
            comp2 = per.tile([128, S // 2], F32, tag="comp2")
            xrz_sb = per.tile([128, 2 * L], F32, tag="xrz")
            xn_sb = per.tile([128, L], F32, tag="xn")

            # ---- Phase 1: compress matmul + gelu -------------------------
            # s-chunk pairs run CONCURRENTLY on the PE via column tiling:
            # even chunk -> array col strips 0-63, odd chunk -> 64-127.
            # The [128, L] psum pair tile is already in comp2's layout.
            with tc.tile_pool(name="resp", bufs=4) as resp, \
                 tc.tile_pool(name="cpsum", bufs=1, space="PSUM") as cpsum:
                ctiles = [cpsum.tile([128, L], F32, tag=f"c{p}", name=f"c{p}") for p in range(4)]
                for dc in range(NDC):
                    rt = resp.tile([128, S], F32, tag="res")
                    # split each 2MiB row-chunk across both HWDGE rings
                    nc.sync.dma_start(
                        rt[:, 0:S // 2],
                        resT.ap()[dc * 128:(dc + 1) * 128, 0:S // 2])
                    nc.scalar.dma_start(
                        rt[:, S // 2:S],
                        resT.ap()[dc * 128:(dc + 1) * 128, S // 2:S])
                    for p in range(4):
                        nc.tensor.matmul(
                            ctiles[p][0:64, :],
                            cw_sb[:, dc * C:(dc + 1) * C],
                            rt[:, (2 * p) * L:(2 * p + 1) * L],
                            start=(dc == 0), stop=(dc == NDC - 1),
                            tile_position=(0, 0),
                        )
                        nc.tensor.matmul(
                            ctiles[p][64:128, :],
                            cw_sb[:, dc * C:(dc + 1) * C],
                            rt[:, (2 * p + 1) * L:(2 * p + 2) * L],
                            start=(dc == 0), stop=(dc == NDC - 1),
                            tile_position=(0, 64),
                        )
                for p in range(4):
                    nc.scalar.activation(
                        comp2[:, L * p:L * (p + 1)],
                        ctiles[p][:], AF.Gelu, bias=cb2_sb,
                    )

            # ---- Phase 2: x-gates directly into blocked layout -----------
            # lhsT = blockdiag([wihT_g, wihT_g]) over the chunk-pair rows of
            # comp2 -> out [32, L] at partition 32j = blocks 2j (rows 0-15)
            # and 2j+1 (rows 16-31).
            with tc.tile_pool(name="xpsum", bufs=1, space="PSUM") as xpsum:
                for g, (wt2, dst, bias) in enumerate([
                    (wih2r_sb, xrz_sb[:, 0:L], brc_sb),
                    (wih2z_sb, xrz_sb[:, L:2 * L], bzc_sb),
                    (wih2n_sb, xn_sb[:], bnc_sb),
                ]):
                    ps = xpsum.tile([128, L], F32, tag=f"xg{g}", name=f"xg{g}")
                    for j in range(4):
                        nc.tensor.matmul(
                            ps[32 * j:32 * j + 32, :], wt2[:],
                            comp2[:, j * L:(j + 1) * L],
                            start=True, stop=True,
                            tile_position=(0, 32 * j),
                        )
                    nc.scalar.activation(
                        dst, ps[:], AF.Identity, bias=bias[:, 0:1],
                    )

            # ---- Phase 3: Picard sweeps ----------------------------------
            # hs tiles hold the SHIFTED trajectory: hs[:, 0] = boundary
            # column (end of previous block = h_{t-1} for the block start),
            # hs[:, 1:L] = h[:, 0:L-1].  Gate matmuls then read hs directly.
            with tc.tile_pool(name="spsum", bufs=1, space="PSUM") as sps, \
                 tc.tile_pool(name="swp", bufs=1) as swp:
                rzps = sps.tile([128, 2 * L], F32, tag="rzps")
                wps = sps.tile([128, L], F32, tag="wps")
                ptps = sps.tile([16, NB], F32, tag="ptps")
                otps = sps.tile([16, NB], F32, tag="otps")
                ebps = sps.tile([128, NB], F32, tag="ebps")

                hs0 = swp.tile([128, L], F32, tag="hs0")
                hs1 = swp.tile([128, L], F32, tag="hs1")
                h_out = swp.tile([128, L], F32, tag="hout")
                s_t = swp.tile([128, 2 * L], F32, tag="st")
                rz = swp.tile([128, 2 * L], F32, tag="rz")
                u_t = swp.tile([128, L], F32, tag="ut")
                v_t = swp.tile([128, L], F32, tag="vt")
                n_t = swp.tile([128, L], F32, tag="nt")
                c_t = swp.tile([128, L], F32, tag="ct")
                P_t = swp.tile([128, L], F32, tag="Pt")
                O_t = swp.tile([128, L], F32, tag="Ot")
                pm = swp.tile([128, NB], F32, tag="pm")
                om = swp.tile([128, NB], F32, tag="om")
                ot_sb = swp.tile([16, NB], F32, tag="otsb")
                hb_sb = swp.tile([16, NB], F32, tag="hbsb")
                eb_sb = swp.tile([128, NB], F32, tag="ebsb")

                hsb0 = swp.tile([128, L], BF16, tag="hsb0")
                hsb1 = swp.tile([128, L], BF16, tag="hsb1")
                hbq = swp.tile([128, 1], F32, tag="hbq")

                nc.vector.memset(hsb0[:], 0.0)
                nc.vector.memset(hs0[:], 0.0)
                hb_cur, hb_nxt = hsb0, hsb1
                h_cur, h_nxt = hs0, hs1
                for k in range(N_SWEEP):
                    # first N_BF sweeps evaluate the gate matmuls in bf16
                    # (1 PE pass instead of fp32's 2); PSUM accumulation and
                    # everything downstream stays fp32.  The fp32 tail
                    # re-converges to the exact fp32 fixpoint.
                    bf_gates = k < N_BF
                    bf_write = k < N_BF - 1
                    if bf_gates:
                        gate_list = ((wrb_sb, rzps[:, 0:L]),
                                     (wzb_sb, rzps[:, L:2 * L]),
                                     (wnb_sb, wps[:]))
                        rhs = hb_cur
                    else:
                        gate_list = ((wr_sb, rzps[:, 0:L]),
                                     (wz_sb, rzps[:, L:2 * L]),
                                     (wn_sb, wps[:]))
                        rhs = h_cur
                    for wt, ps in gate_list:
                        nc.tensor.matmul(
                            ps[:], wt, rhs[:], start=True, stop=True)
                    nc.vector.tensor_tensor(s_t[:], rzps[:], xrz_sb[:], OP.add)
                    nc.scalar.activation(rz[:], s_t[:], AF.Sigmoid)
                    nc.vector.scalar_tensor_tensor(
                        u_t[:], wps[:], bhn_sb, rz[:, 0:L], OP.add, OP.mult)
                    nc.vector.tensor_tensor(v_t[:], u_t[:], xn_sb[:], OP.add)
                    nc.scalar.activation(n_t[:], v_t[:], AF.Tanh)
                    nc.vector.scalar_tensor_tensor(
                        c_t[:], rz[:, L:2 * L], 1.0, n_t[:], OP.subtract, OP.mult)
                    nc.vector.tensor_tensor_scan(
                        O_t[:], rz[:, L:2 * L], c_t[:], 0.0, OP.mult, OP.subtract)
                    nc.vector.tensor_tensor_scan(
                        P_t[:], rz[:, L:2 * L], ones_sb, 1.0, OP.mult, OP.mult)
                    # boundary chain across the 8 blocks
                    nc.vector.tensor_scalar(
                        pm[:], mb_sb, P_t[:, L - 1:L], None, OP.mult)
                    nc.vector.tensor_scalar(
                        om[:], mb_sb, O_t[:, L - 1:L], None, OP.mult)
                    nc.tensor.matmul(ptps[:], g_sb, pm[:], start=True, stop=True)
                    nc.tensor.matmul(otps[:], g_sb, om[:], start=True, stop=True)
                    nc.vector.tensor_copy(ot_sb[:], otps[:])
                    nc.vector.tensor_tensor_scan(
                        hb_sb[:], ptps[:], ot_sb[:], 0.0, OP.mult, OP.add)
                    nc.tensor.matmul(ebps[:], s_sb, hb_sb[:], start=True, stop=True)
                    nc.vector.tensor_tensor(eb_sb[:], ebps[:], sel_sb, OP.mult)
                    # boundary column -> hs_nxt[:, 0], shifted combine -> 1:L
                    nc.vector.tensor_reduce(hbq[:], eb_sb[:], AX.X, OP.add)
                    if bf_write:
                        nc.vector.tensor_copy(hb_nxt[:, 0:1], hbq[:])
                        nc.vector.scalar_tensor_tensor(
                            hb_nxt[:, 1:L], P_t[:, 0:L - 1], hbq[:, 0:1],
                            O_t[:, 0:L - 1], OP.mult, OP.add)
                        hb_cur, hb_nxt = hb_nxt, hb_cur
                    else:
                        nc.vector.tensor_copy(h_nxt[:, 0:1], hbq[:])
                        nc.vector.scalar_tensor_tensor(
                            h_nxt[:, 1:L], P_t[:, 0:L - 1], hbq[:, 0:1],
                            O_t[:, 0:L - 1], OP.mult, OP.add)
                        if k == N_SWEEP - 1:
                            nc.vector.scalar_tensor_tensor(
                                h_out[:], P_t[:], hbq[:, 0:1], O_t[:],
                                OP.mult, OP.add)
                        h_cur, h_nxt = h_nxt, h_cur

                nc.sync.dma_start(out.ap(), h_out[:])

    return nc


_NC_CACHE = None


def kernel(residual, compress_w, compress_b, w_ih, w_hh, b_ih, b_hh):
    global _NC_CACHE
    _install_bir_fix()
    from concourse.bass_utils import run_bass_kernel_spmd

    f32 = np.float32
    residual = np.ascontiguousarray(residual, dtype=f32)
    compress_w = np.asarray(compress_w, dtype=f32)
    compress_b = np.asarray(compress_b, dtype=f32)
    w_ih = np.asarray(w_ih, dtype=f32)
    w_hh = np.asarray(w_hh, dtype=f32)
    b_ih = np.asarray(b_ih, dtype=f32)
    b_hh = np.asarray(b_hh, dtype=f32)

    # host-side shared weight prep (layout only)
    cwT = np.ascontiguousarray(compress_w.T)                      # [D, C]
    cw_tiles = np.ascontiguousarray(
        cwT.reshape(D // 128, 128, C).transpose(1, 0, 2).reshape(128, -1))
    wihT = np.ascontiguousarray(w_ih.T)                           # [C, 3H]

    def wih2(g):   # [128, 32] blockdiag over chunk-pair rows
        m = np.zeros((128, 32), f32)
        m[0:C, 0:16] = wihT[:, 16 * g:16 * g + 16]
        m[C:2 * C, 16:32] = wihT[:, 16 * g:16 * g + 16]
        return m

    def blockdiag_tiled(w):   # w: [H, H] -> [128, 128] (8 diagonal blocks)
        return np.ascontiguousarray(np.kron(np.eye(NB, dtype=f32), w.T.astype(f32)))

    wr_t = blockdiag_tiled(w_hh[:H])
    wz_t = blockdiag_tiled(w_hh[H:2 * H])
    wn_t = blockdiag_tiled(w_hh[2 * H:])

    g_np = np.zeros((128, 16), f32)
    for kk in range(128):
        g_np[kk, kk % 16] = 1.0
    s_np = np.ascontiguousarray(g_np.T)
    maskb_np = np.zeros((128, NB), f32)
    sel_np = np.zeros((128, NB), f32)
    for kk in range(128):
        maskb_np[kk, kk // 16] = 1.0
        if kk // 16 >= 1:
            sel_np[kk, kk // 16 - 1] = 1.0

    smat_pad = np.zeros((128, 128), f32)
    smat_pad[0:16, :] = s_np
    consts_np = np.concatenate([
        cw_tiles, wr_t, wz_t, wn_t, wih2(0), wih2(1), wih2(2), g_np, smat_pad,
        maskb_np, sel_np,
        np.tile(compress_b, 2).reshape(128, 1),
        np.tile(b_ih[:H] + b_hh[:H], NB).reshape(128, 1),
        np.tile(b_ih[H:2 * H] + b_hh[H:2 * H], NB).reshape(128, 1),
        np.tile(b_ih[2 * H:], NB).reshape(128, 1),
        np.tile(b_hh[2 * H:], NB).reshape(128, 1),
    ], axis=1).astype(f32)
    import ml_dtypes
    wbf_np = np.ascontiguousarray(
        np.concatenate([wr_t, wz_t, wn_t], axis=1).astype(ml_dtypes.bfloat16))
    shared = {"consts": np.ascontiguousarray(consts_np), "wbf": wbf_np}

    in_maps = []
    for b in range(NCORES):
        m = dict(shared)
        m["resT"] = np.ascontiguousarray(residual[b].T)
        in_maps.append(m)

    if _NC_CACHE is None:
        _NC_CACHE = _build_nc()
    nc = _NC_CACHE

    res = run_bass_kernel_spmd(nc, in_maps, core_ids=list(range(NCORES)))
    if res.exec_time_ns is not None:
        print(f"HW exec time: {res.exec_time_ns} ns")

    out = np.zeros((B, S, H), f32)
    for b in range(NCORES):
        hb = res.results[b]["out"]                     # [128, L] blocked
        out[b] = hb.reshape(NB, H, L).transpose(0, 2, 1).reshape(S, H)
    return out
